# revision 22
# baseline (speedup 1.0000x reference)
"""Mixtral attention layer (B=1, S=2048, H=4096, NH=32, NKV=8, HD=128) on 8
Trainium2 NeuronCores, tensor-parallel over heads.

Sharding: core c owns 4 query heads + 1 KV head (column-shard of wq/wk/wv,
row-shard of wo).  Each core computes a full [S, H] partial of the o_proj
output; the host sums the 8 partials and adds the residual (the gather of a
row-parallel matmul).

Per-core pipeline (projection/attention matmuls in float32r = fp22-truncated
fp32, full PE rate at N>=256):
  Phase 1 (fused stats + projections): x^T streamed once in fp32r (2-chunk
    group DMAs); per H-chunk, ACT squares x into bf16 and a ones-vector
    matmul accumulates sum(x^2) over H in PSUM alongside the 6 projection
    matmuls (4 q heads + k + v).  r = 1/sqrt(mean+eps) is partition-broadcast
    and folded into full-width RoPE cos/sin tables; PSUM evacuation applies
    norm + RoPE on DVE.  norm_w is folded into the weights on the host.
    Weights are loaded in chunks (wq x4, wk/wv x2) so the first projection
    matmuls start as soon as the first chunks land.
  Attention: per head-pair sweep (both heads share this core's single KV
    head - GQA), causal flash-style: scores^T = k^T.T @ q^T chunkwise
    (narrowed to the valid column range on diagonal chunks), exp on ACT
    (PSUM->SBUF), causal mask via affine_select on the 128-wide diagonal
    block only, unnormalized AV + ones-matmul row-sum Z accumulate in PSUM;
    1/Z applied at AV evacuation into SBUF-resident attn^T.
  o_proj: per i-tile, attn^T @ wo chunked ht-pair-outer; wo is DMA'd in 4
    column chunks so the first matmuls don't wait for the full 8MB load.
    Output DMAs batched per (sc, ht-pair) and issued on the gpsimd SWDGE
    queue to keep them off the HWDGE descriptor path.

q^T is spilled to internal DRAM between phases (SBUF pressure, via the
Activation HWDGE queue); attn^T reuses the wk/wv SBUF slots after the
projections retire.
"""

import math

import numpy as np

import concourse.bass as bass
import concourse.tile as tile
from concourse import bacc, mybir
from concourse.masks import make_identity

F32 = mybir.dt.float32
F32R = mybir.dt.float32r
BF16 = mybir.dt.bfloat16

# Full problem dims
B, S, H, NH, NKV, HD = 1, 2048, 4096, 32, 8, 128
EPS = 1e-5
N_CORES = 8
QH = NH // N_CORES          # query heads per core = 4
DQ = QH * HD                # q columns per core = 512
DKV = (NKV // N_CORES) * HD  # kv columns per core = 128


def build_bass(s=S, h=H, qh=QH, stop_after=None, diag=None):
    """Build the single-core Bass module (same NEFF on all 8 cores)."""
    ST = 512 if s >= 512 else s       # s-tile width (proj + attention i-tiles)
    NST = s // ST                     # number of s-tiles
    HC = h // 128                     # H contraction chunks
    NJ = s // 128                     # j chunks (keys)
    dq = qh * HD
    scale = 1.0 / math.sqrt(HD)
    XG = 2                            # x chunks per DMA group
    WQG = HC // 8                     # wq chunks per DMA group
    WKG = HC // 2                     # wk/wv chunks per DMA group
    HTP = 1024                        # o_proj ht-pair width (2 PSUM banks)
    NHP = h // HTP
    WOC = 512                         # wo load chunk width

    nc = bacc.Bacc(None, target_bir_lowering=False)

    xT = nc.dram_tensor("xT", [h, s], F32R, kind="ExternalInput")
    wq = nc.dram_tensor("wq", [h, dq], F32R, kind="ExternalInput")
    wk = nc.dram_tensor("wk", [h, DKV], F32R, kind="ExternalInput")
    wv = nc.dram_tensor("wv", [h, DKV], F32R, kind="ExternalInput")
    wo = nc.dram_tensor("wo", [dq, h], F32R, kind="ExternalInput")
    cosT = nc.dram_tensor("cosT", [HD, s], F32, kind="ExternalInput")
    sinTs = nc.dram_tensor("sinTs", [HD, s], F32, kind="ExternalInput")
    out = nc.dram_tensor("out", [s, h], F32, kind="ExternalOutput")

    xT_t = xT.rearrange("(ho hi) s -> hi ho s", hi=128)
    wq_t = wq.rearrange("(ho hi) d -> hi ho d", hi=128)
    wk_t = wk.rearrange("(ho hi) d -> hi ho d", hi=128)
    wv_t = wv.rearrange("(ho hi) d -> hi ho d", hi=128)
    wo_t = wo.rearrange("(do di) h -> di do h", di=128)

    with tile.TileContext(nc) as tc:
        with (
            tc.tile_pool(name="persist", bufs=1) as persist,
            tc.tile_pool(name="xin", bufs=3) as xin,
            tc.tile_pool(name="x2b", bufs=1) as x2b,
            tc.tile_pool(name="rope", bufs=3) as ropep,
            tc.tile_pool(name="statp", bufs=2) as statp,
            tc.tile_pool(name="tabp", bufs=2) as tabp,
            tc.tile_pool(name="tabin", bufs=1) as tabin,
            tc.tile_pool(name="bcastp", bufs=1) as bcastp,
            tc.tile_pool(name="probs", bufs=3) as probs,
            tc.tile_pool(name="outp", bufs=2) as outp,
            tc.tile_pool(name="qin", bufs=4) as qin,
            tc.tile_pool(name="dramp", bufs=1, space="DRAM") as dramp,
            tc.tile_pool(name="acc_ps", bufs=8, space="PSUM") as acc_ps,
        ):
            # ---- persistent SBUF tensors ----
            # Slot reuse chains (same tag, sequential lifetimes):
            #   wq (8MB) -> wo (8MB)         tag "bigw"
            #   wk (2MB) -> attnT heads 0-1  tag "wk"
            #   wv (2MB) -> attnT heads 2-3  tag "wv"
            #   cos (1MB) -> v natural (1MB) tag "cosvnat"
            wq_sb = persist.tile([128, HC, dq], F32R, tag="bigw")
            wk_sb = persist.tile([128, HC, DKV], F32R, tag="wk")
            wv_sb = persist.tile([128, HC, DKV], F32R, tag="wv")
            ones_f = persist.tile([128, 1], F32, tag="ones_f")
            ones_sb = persist.tile([128, 1], F32R, tag="ones")
            ones_bf = persist.tile([128, 1], BF16, tag="ones_bf")
            eps_sb = persist.tile([1, 1], F32, tag="eps")
            ident_sb = persist.tile([128, 128], F32, tag="ident")
            kT_sb = persist.tile([128, s], F32R, tag="kT")
            vT_sb = persist.tile([128, s], F32, tag="vT")
            vnat_sb = persist.tile([128, NJ, 128], BF16, tag="vnat")
            # q^T spilled to DRAM, re-streamed by attention
            qT_dr = dramp.tile([128, qh, s], F32R, tag="qT_dr")

            # chunked weight loads: the first two x groups of tile 0 are
            # requested ahead of the weights so the DMA FIFO serves them
            # first; wk/wv before wq (k/v matmuls are ordered first per
            # chunk), wq in small chunks so early q matmuls start fast
            x_pre = []
            for hg in range(3):
                x_sb = xin.tile([128, XG, ST], F32R, name=f"x_pre{hg}")
                nc.sync.dma_start(out=x_sb,
                                  in_=xT_t[:, hg * XG:(hg + 1) * XG,
                                           bass.ts(0, ST)])
                x_pre.append(x_sb)
            def load_wq(g):
                nc.scalar.dma_start(out=wq_sb[:, g * WQG:(g + 1) * WQG, :],
                                    in_=wq_t[:, g * WQG:(g + 1) * WQG, :])

            def load_wkv(g):
                nc.scalar.dma_start(out=wk_sb[:, g * WKG:(g + 1) * WKG, :],
                                    in_=wk_t[:, g * WKG:(g + 1) * WKG, :])
                nc.scalar.dma_start(out=wv_sb[:, g * WKG:(g + 1) * WKG, :],
                                    in_=wv_t[:, g * WKG:(g + 1) * WKG, :])

            load_wq(0)
            load_wkv(0)
            load_wq(1)
            # remaining weight-chunk dispatches are spread through tile 0's
            # loop (emitted from pass_b) so the in-order ACT SEQ never sits
            # on a full DMA request queue ahead of the squares
            pending_loads = [lambda: load_wq(2),
                             lambda: load_wkv(1), lambda: load_wq(3),
                             lambda: load_wq(4), lambda: load_wq(5),
                             lambda: load_wq(6), lambda: load_wq(7),
                             ]
            nc.vector.memset(ones_f, 1.0)
            nc.scalar.copy(ones_sb, ones_f)
            nc.scalar.copy(ones_bf, ones_f)
            nc.vector.memset(eps_sb, EPS)
            make_identity(nc, ident_sb)

            q_pref = {}

            # ---- phase 1: fused norm stats + q/k/v projections, one pass
            # over x^T in fp32r ----
            def pass_b(st):
                ss = bass.ts(st, ST)
                cs_sb = tabin.tile([128, ST], F32, tag="cosin", name="cs_sb")
                nc.scalar.dma_start(out=cs_sb, in_=cosT[:, ss])
                sn_sb = tabin.tile([128, ST], F32, tag="sinin", name="sn_sb")
                nc.scalar.dma_start(out=sn_sb, in_=sinTs[:, ss])
                sq_ps = acc_ps.tile([1, ST], F32, tag="acc", name="sq_ps")
                q_ps = [acc_ps.tile([128, ST], F32, tag="acc", name=f"q_ps{m}")
                        for m in range(qh)]
                k_ps = acc_ps.tile([128, ST], F32, tag="acc", name="k_ps")
                v_ps = acc_ps.tile([128, ST], F32, tag="acc", name="v_ps")
                for hg in range(HC // XG):
                    if st == 0 and hg >= 1 and pending_loads:
                        pending_loads.pop(0)()
                    if st == NST - 1 and hg == 4 and stop_after is None:
                        for hh in range(2):
                            q_sb = qin.tile([128, ST], F32R, tag="q",
                                            name=f"q_pref{hh}", bufs=2)
                            nc.scalar.dma_start(
                                out=q_sb, in_=qT_dr[:, hh, bass.ts(0, ST)])
                            q_pref[hh] = q_sb
                    if st == 0 and hg < 3:
                        x_sb = x_pre[hg]
                    else:
                        x_sb = xin.tile([128, XG, ST], F32R)
                        nc.sync.dma_start(out=x_sb,
                                          in_=xT_t[:, hg * XG:(hg + 1) * XG,
                                                   ss])
                    for hi in range(XG):
                        hc = hg * XG + hi
                        xs = x_sb[:, hi, :]
                        st_, sp_ = (hc == 0), (hc == HC - 1)
                        x2_sb = x2b.tile([128, ST], BF16)
                        if hc % 2 == 0:
                            nc.scalar.square(x2_sb, xs)
                        else:
                            nc.gpsimd.tensor_mul(x2_sb, xs, xs)
                        nc.tensor.matmul(sq_ps, ones_bf, x2_sb,
                                         start=st_, stop=sp_)
                        for m in range(qh):
                            nc.tensor.matmul(
                                q_ps[m], wq_sb[:, hc, bass.ts(m, 128)], xs,
                                start=st_, stop=sp_,
                            )
                        nc.tensor.matmul(k_ps, wk_sb[:, hc, :], xs,
                                         start=st_, stop=sp_)
                        nc.tensor.matmul(v_ps, wv_sb[:, hc, :], xs,
                                         start=st_, stop=sp_)
                # r = 1/sqrt(mean + eps); fold into cos/sin tables
                sd_sb = statp.tile([1, ST], F32, tag="stat", name="sd_sb")
                nc.scalar.activation(
                    sd_sb, sq_ps, mybir.ActivationFunctionType.Sqrt,
                    bias=eps_sb, scale=1.0 / h,
                )
                rr_sb = statp.tile([1, ST], F32, tag="stat", name="rr_sb")
                nc.vector.reciprocal(rr_sb, sd_sb)
                R_t = tabp.tile([128, ST], F32, tag="R", name="R_t",
                                bufs=1)
                nc.gpsimd.partition_broadcast(R_t, rr_sb)
                cp_t = tabp.tile([128, ST], F32, tag="cp", name="cp_t",
                                 bufs=1)
                nc.vector.tensor_mul(cp_t, cs_sb, R_t)
                sp_t = tabp.tile([128, ST], F32, tag="sp", name="sp_t",
                                 bufs=1)
                nc.vector.tensor_mul(sp_t, sn_sb, R_t)

                # evacuation: fast ACT copy frees the PSUM bank, then
                # norm+RoPE happens SBUF-side on DVE (in place; the u-halves
                # read the raw values before the cos-multiply overwrites)
                def rope_xform(dst):
                    u_sb = ropep.tile([128, ST], F32, tag="u", name="u_sb",
                                      bufs=1)
                    nc.vector.tensor_mul(
                        u_sb[0:64, :], dst[64:128, :], sp_t[64:128, :])
                    nc.vector.tensor_mul(
                        u_sb[64:128, :], dst[0:64, :], sp_t[0:64, :])
                    nc.vector.tensor_mul(dst, dst, cp_t)
                    nc.vector.tensor_add(dst, dst, u_sb)

                if diag == "no_evac":
                    return
                nc.scalar.copy(kT_sb[:, ss], k_ps)
                nc.vector.tensor_copy(vT_sb[:, ss], v_ps)
                nc.vector.tensor_mul(vT_sb[:, ss], vT_sb[:, ss], R_t)
                for jc in range(st * (ST // 128), (st + 1) * (ST // 128)):
                    vt_ps = acc_ps.tile([128, 128], F32, tag="acc",
                                        name="vt_ps")
                    nc.tensor.transpose(vt_ps, vT_sb[:, bass.ts(jc, 128)],
                                        ident_sb)
                    nc.scalar.copy(vnat_sb[:, jc, :], vt_ps)
                rope_xform(kT_sb[:, ss])
                for m in range(qh):
                    dst = ropep.tile([128, ST], F32R, tag="t", name="t_sb",
                                     bufs=2)
                    nc.scalar.copy(dst, q_ps[m])
                    rope_xform(dst)
                    nc.scalar.dma_start(out=qT_dr[:, m, ss], in_=dst)

            for st in range(NST):
                pass_b(st)

            # ---- phase 2: wo load (sync queue: idle at the transition) ----
            wo_sb = persist.tile([128, qh, h], F32R, tag="bigw")
            if stop_after != "p1":
                for wc in range(h // WOC):
                    nc.sync.dma_start(
                        out=wo_sb[:, :, bass.ts(wc, WOC)],
                        in_=wo_t[:, :, bass.ts(wc, WOC)],
                    )

            # attn^T reuses the wk/wv slots (heads 0-1 / 2-3)
            attnT_h = [
                persist.tile([128, 2, s], F32R, tag="wk", name="attnT01"),
                persist.tile([128, 2, s], F32R, tag="wv", name="attnT23"),
            ]

            def attn_slice(m, sl):
                return attnT_h[m // 2][:, m % 2, sl]

            # ---- phase 3 + 4 interleaved: attention per i-tile (both
            # head pairs), then immediately the o_proj matmuls for that
            # i-range so they fill PE stalls in the next i-tile's attention
            def attn_tile(hp, ti, q_all):
                heads = (2 * hp, 2 * hp + 1)
                iss = bass.ts(ti, ST)
                q_sbs = [q_all[2 * hp], q_all[2 * hp + 1]]
                av_ps = [acc_ps.tile([128, ST], F32, tag="acc",
                                     name=f"av_ps{i}") for i in range(2)]
                z_ps = [acc_ps.tile([1, ST], F32, tag="acc",
                                    name=f"z_ps{i}") for i in range(2)]
                njc = (ti + 1) * (ST // 128)
                for jc in range(njc):
                    st_, sp_ = (jc == 0), (jc == njc - 1)
                    # diagonal chunks: columns left of the block are fully
                    # masked -> compute scores/exp only on [dcol, ST),
                    # zero the left part, mask the 128-wide block
                    dcol = min(max(0, jc * 128 - ti * ST), ST - 256)
                    w = ST - dcol
                    for i in range(2):
                        s_ps = acc_ps.tile([128, w], F32, tag="acc",
                                           name=f"s_ps{i}")
                        nc.tensor.matmul(
                            s_ps, kT_sb[:, bass.ts(jc, 128)],
                            q_sbs[i][:, dcol:],
                            start=True, stop=True,
                        )
                        p_sb = probs.tile([128, ST], BF16, tag="p",
                                          name=f"p_sb{i}", bufs=3)
                        if dcol > 0:
                            nc.vector.memset(p_sb[:, 0:dcol], 0.0)
                        nc.scalar.activation(
                            p_sb[:, dcol:], s_ps,
                            mybir.ActivationFunctionType.Exp,
                            scale=scale,
                        )
                        if (jc + 1) * 128 > ti * ST:
                            nc.gpsimd.affine_select(
                                out=p_sb[:, dcol:dcol + 128],
                                in_=p_sb[:, dcol:dcol + 128],
                                pattern=[[1, 128]],
                                compare_op=mybir.AluOpType.is_ge,
                                fill=0.0,
                                base=0,
                                channel_multiplier=-1,
                            )
                        nc.tensor.matmul(av_ps[i], vnat_sb[:, jc, :], p_sb,
                                         start=st_, stop=sp_)
                        nc.tensor.matmul(z_ps[i], ones_bf, p_sb,
                                         start=st_, stop=sp_)
                for i, hh in enumerate(heads):
                    zr_sb = statp.tile([1, ST], F32, tag="stat",
                                       name="zr_sb")
                    nc.vector.reciprocal(zr_sb, z_ps[i])
                    ZR_sb = bcastp.tile([128, ST], F32, tag="bcast",
                                        name="ZR_sb")
                    nc.gpsimd.partition_broadcast(ZR_sb, zr_sb)
                    nc.vector.tensor_mul(attn_slice(hh, iss), av_ps[i],
                                         ZR_sb)

            def o_proj_tile(ti):
                # per i-tile: sc-outer, ht-pair inner; 2 PSUM banks per
                # pair, evacuate into one [128, HTP] tile, single DMA out
                # on the gpsimd SWDGE queue
                for hp in range(NHP):
                    for sc in range(ti * (ST // 128), (ti + 1) * (ST // 128)):
                        scs = bass.ts(sc, 128)
                        o_ps = [acc_ps.tile([128, 512], F32, tag="acc",
                                            name=f"o_ps{_hh}")
                                for _hh in range(2)]
                        for hh in range(2):
                            for m in range(qh):
                                nc.tensor.matmul(
                                    o_ps[hh], attn_slice(m, scs),
                                    wo_sb[:, m, bass.ts(2 * hp + hh, 512)],
                                    start=(m == 0), stop=(m == qh - 1),
                                )
                        o_sb = outp.tile([128, 2, 512], F32, name="o_sb",
                                         bufs=3)
                        for hh in range(2):
                            if (sc + hp + hh) % 2 == 0:
                                nc.scalar.copy(o_sb[:, hh, :], o_ps[hh])
                            else:
                                nc.vector.tensor_copy(o_sb[:, hh, :], o_ps[hh])
                        if ti == NST - 1 and sc % 4 == 3:
                            engs = [nc.sync, nc.scalar, nc.gpsimd]
                            for hh in range(2):
                                engs[(2 * hp + hh) % 3].dma_start(
                                    out=out[scs, bass.ts(2 * hp + hh, 512)],
                                    in_=o_sb[:, hh, :],
                                )
                        else:
                            eng = (nc.sync if (sc + hp) % 2 == 0
                                   else nc.gpsimd)
                            eng.dma_start(
                                out=out[scs, bass.ts(hp, HTP)], in_=o_sb
                            )

            if stop_after not in ("p1", "p2"):
                def attn_full_tile(ti):
                    iss = bass.ts(ti, ST)
                    q_all = []
                    for hh in range(qh):
                        if ti == 0 and hh in q_pref:
                            q_all.append(q_pref[hh])
                            continue
                        q_sb = qin.tile([128, ST], F32R, tag="q",
                                        name=f"q_sb{hh}", bufs=2)
                        nc.scalar.dma_start(out=q_sb,
                                            in_=qT_dr[:, hh, iss])
                        q_all.append(q_sb)
                    for hp in range(qh // 2):
                        attn_tile(hp, ti, q_all)

                attn_full_tile(0)
                for ti in range(NST):
                    if ti + 1 < NST:
                        attn_full_tile(ti + 1)
                    if stop_after is None:
                        o_proj_tile(ti)

    nc.compile()
    return nc


def make_core_inputs(hidden_states, cos, sin, norm_w, wq, wk, wv, wo,
                     s=S, h=H, qh=QH, n_cores=N_CORES):
    """Host-side sharding + layout preparation. Returns list of in_maps."""
    dq = qh * HD
    dkv = DKV
    x = np.asarray(hidden_states, dtype=np.float32).reshape(s, h)
    nw = np.asarray(norm_w, dtype=np.float32)
    xT = np.ascontiguousarray(x.T)                      # [h, s]
    cosT = np.ascontiguousarray(np.asarray(cos, np.float32).reshape(s, HD).T)
    sinT = np.ascontiguousarray(np.asarray(sin, np.float32).reshape(s, HD).T)
    # swapped/sign-flipped sin table: rows 0:64 = +sin_half, 64:128 = -sin_half
    sin_half = sinT[0:64]
    sinTs = np.ascontiguousarray(np.concatenate([sinT[64:128], -sin_half], axis=0))
    # fold norm_w into the projection weights
    wq_f = np.asarray(wq, np.float32) * nw[:, None]
    wk_f = np.asarray(wk, np.float32) * nw[:, None]
    wv_f = np.asarray(wv, np.float32) * nw[:, None]
    wo_f = np.asarray(wo, np.float32)

    in_maps = []
    for c in range(n_cores):
        in_maps.append({
            "xT": xT,
            "wq": np.ascontiguousarray(wq_f[:, c * dq:(c + 1) * dq]),
            "wk": np.ascontiguousarray(wk_f[:, c * dkv:(c + 1) * dkv]),
            "wv": np.ascontiguousarray(wv_f[:, c * dkv:(c + 1) * dkv]),
            "wo": np.ascontiguousarray(wo_f[c * dq:(c + 1) * dq, :]),
            "cosT": cosT,
            "sinTs": sinTs,
        })
    return in_maps


_NC_CACHE = {}


def kernel(hidden_states, cos, sin, norm_w, wq, wk, wv, wo):
    from concourse.bass_utils import run_bass_kernel_spmd

    if "nc" not in _NC_CACHE:
        _NC_CACHE["nc"] = build_bass()
    nc = _NC_CACHE["nc"]
    in_maps = make_core_inputs(hidden_states, cos, sin, norm_w, wq, wk, wv, wo)
    res = run_bass_kernel_spmd(nc, in_maps, core_ids=list(range(N_CORES)))
    partials = [m["out"] for m in res.results]
    out = np.asarray(hidden_states, np.float32).reshape(S, H).copy()
    for p in partials:
        out += p
    return out.reshape(B, S, H)


# revision 47
# speedup vs baseline: 1.1617x; 1.1617x over previous
"""Mixtral attention layer (B=1, S=2048, H=4096, NH=32, NKV=8, HD=128) on 8
Trainium2 NeuronCores, tensor-parallel over heads.

Sharding: core c owns 4 query heads + 1 KV head (column-shard of wq/wk/wv,
row-shard of wo).  Each core computes a full [S, H] partial of the o_proj
output; the host sums the 8 partials and adds the residual (the gather of a
row-parallel matmul).

Per-core pipeline (projection/attention matmuls in float32r = fp22-truncated
fp32, full PE rate at N>=256):
  Phase 1 (fused stats + projections): x^T streamed once in fp32r (2-chunk
    group DMAs); per H-chunk, ACT squares x into bf16 and a ones-vector
    matmul accumulates sum(x^2) over H in PSUM alongside the 6 projection
    matmuls (4 q heads + k + v).  r = 1/sqrt(mean+eps) is partition-broadcast
    and folded into full-width RoPE cos/sin tables; PSUM evacuation applies
    norm + RoPE on DVE.  norm_w is folded into the weights on the host.
    Weights are loaded in chunks (wq x4, wk/wv x2) so the first projection
    matmuls start as soon as the first chunks land.
  Attention: per head-pair sweep (both heads share this core's single KV
    head - GQA), causal flash-style: scores^T = k^T.T @ q^T chunkwise
    (narrowed to the valid column range on diagonal chunks), exp on ACT
    (PSUM->SBUF), causal mask via affine_select on the 128-wide diagonal
    block only, unnormalized AV + ones-matmul row-sum Z accumulate in PSUM;
    1/Z applied at AV evacuation into SBUF-resident attn^T.
  o_proj: per i-tile, attn^T @ wo chunked ht-pair-outer; wo is DMA'd in 4
    column chunks so the first matmuls don't wait for the full 8MB load.
    Output DMAs batched per (sc, ht-pair) and issued on the gpsimd SWDGE
    queue to keep them off the HWDGE descriptor path.

q^T is spilled to internal DRAM between phases (SBUF pressure, via the
Activation HWDGE queue); attn^T reuses the wk/wv SBUF slots after the
projections retire.
"""

import math

import numpy as np

import concourse.bass as bass
import concourse.tile as tile
from concourse import bacc, mybir
from concourse.masks import make_identity

F32 = mybir.dt.float32
F32R = mybir.dt.float32r
BF16 = mybir.dt.bfloat16

# Full problem dims
B, S, H, NH, NKV, HD = 1, 2048, 4096, 32, 8, 128
EPS = 1e-5
N_CORES = 8
QH = NH // N_CORES          # query heads per core = 4
DQ = QH * HD                # q columns per core = 512
DKV = (NKV // N_CORES) * HD  # kv columns per core = 128


def build_bass(s=S, h=H, qh=QH, stop_after=None, diag=None):
    """Build the single-core Bass module (same NEFF on all 8 cores)."""
    ST = 512 if s >= 512 else s       # s-tile width (proj + attention i-tiles)
    NST = s // ST                     # number of s-tiles
    HC = h // 128                     # H contraction chunks
    NJ = s // 128                     # j chunks (keys)
    dq = qh * HD
    scale = 1.0 / math.sqrt(HD)
    XG = 2                            # x chunks per DMA group
    WQG = max(1, HC // 8)             # wq chunks per DMA group
    WKG = max(1, HC // 2)             # wk/wv chunks per DMA group
    NHB = h // 512                    # o_proj 512-col blocks
    NHP = (NHB + 1) // 2              # block pairs per sc chunk
    HTP = 1024                        # o_proj ht-pair width (2 PSUM banks)
    WOC = 512                         # wo load chunk width

    nc = bacc.Bacc(None, target_bir_lowering=False)

    xT = nc.dram_tensor("xT", [h, s], BF16, kind="ExternalInput")
    wq = nc.dram_tensor("wq", [h, dq], BF16, kind="ExternalInput")
    wk = nc.dram_tensor("wk", [h, DKV], BF16, kind="ExternalInput")
    wv = nc.dram_tensor("wv", [h, DKV], BF16, kind="ExternalInput")
    wo = nc.dram_tensor("wo", [dq, h], BF16, kind="ExternalInput")
    cosT = nc.dram_tensor("cosT", [HD, s], F32, kind="ExternalInput")
    sinTs = nc.dram_tensor("sinTs", [HD, s], F32, kind="ExternalInput")
    out = nc.dram_tensor("out", [s, h], F32, kind="ExternalOutput")

    xT_t = xT.rearrange("(ho hi) s -> hi ho s", hi=128)
    wq_t = wq.rearrange("(ho hi) d -> hi ho d", hi=128)
    wk_t = wk.rearrange("(ho hi) d -> hi ho d", hi=128)
    wv_t = wv.rearrange("(ho hi) d -> hi ho d", hi=128)
    wo_t = wo.rearrange("(do di) h -> di do h", di=128)

    with tile.TileContext(nc) as tc:
        with (
            tc.tile_pool(name="persist", bufs=1) as persist,
            tc.tile_pool(name="xin", bufs=4) as xin,
                        tc.tile_pool(name="rope", bufs=3) as ropep,
            tc.tile_pool(name="statp", bufs=2) as statp,
            tc.tile_pool(name="tabp", bufs=2) as tabp,
            tc.tile_pool(name="tabin", bufs=1) as tabin,
            tc.tile_pool(name="outp", bufs=4) as outp,
            tc.tile_pool(name="bcastp", bufs=1) as bcastp,
            tc.tile_pool(name="probs", bufs=6) as probs,
            tc.tile_pool(name="acc_ps", bufs=8, space="PSUM") as acc_ps,
        ):
            # ---- persistent SBUF tensors ----
            # Slot reuse chains (same tag, sequential lifetimes):
            #   wq (8MB) -> wo (8MB)         tag "bigw"
            #   wk (2MB) -> attnT heads 0-1  tag "wk"
            #   wv (2MB) -> attnT heads 2-3  tag "wv"
            #   cos (1MB) -> v natural (1MB) tag "cosvnat"
            wq_sb = persist.tile([128, HC, dq], BF16, tag="bigw")
            wk_sb = persist.tile([128, HC, DKV], BF16, tag="wk")
            wv_sb = persist.tile([128, HC, DKV], BF16, tag="wv")
            ones_f = persist.tile([128, 1], F32, tag="ones_f")
            ones_sb = persist.tile([128, 1], F32R, tag="ones")
            ones_bf = persist.tile([128, 1], BF16, tag="ones_bf")
            eps_sb = persist.tile([1, 1], F32, tag="eps")
            ident_sb = persist.tile([128, 128], F32, tag="ident")
            kT_sb = persist.tile([128, s], BF16, tag="kT")
            vnat_sb = persist.tile([128, NJ, 128], BF16, tag="vnat")
            # q^T stays SBUF-resident in bf16 (scores run in bf16)
            qT_sb = persist.tile([128, qh, s], BF16, tag="qT")

            # chunked weight loads: the first two x groups of tile 0 are
            # requested ahead of the weights so the DMA FIFO serves them
            # first; wk/wv before wq (k/v matmuls are ordered first per
            # chunk), wq in small chunks so early q matmuls start fast
            x_pre = []
            for hg in range(min(3, HC // XG)):
                x_sb = xin.tile([128, XG, ST], BF16, tag="x",
                                name=f"x_pre{hg}", bufs=4)
                nc.sync.dma_start(out=x_sb,
                                  in_=xT_t[:, hg * XG:(hg + 1) * XG,
                                           bass.ts(0, ST)])
                x_pre.append(x_sb)
            def load_wq(g):
                nc.sync.dma_start(out=wq_sb[:, g * WQG:(g + 1) * WQG, :],
                                  in_=wq_t[:, g * WQG:(g + 1) * WQG, :])

            def load_wkv(g):
                nc.sync.dma_start(out=wk_sb[:, g * WKG:(g + 1) * WKG, :],
                                  in_=wk_t[:, g * WKG:(g + 1) * WKG, :])
                nc.sync.dma_start(out=wv_sb[:, g * WKG:(g + 1) * WKG, :],
                                  in_=wv_t[:, g * WKG:(g + 1) * WKG, :])

            n_wq = HC // WQG
            load_wq(0)
            load_wkv(0)
            if n_wq > 1:
                load_wq(1)
            # remaining weight-chunk dispatches are spread through tile 0's
            # loop (emitted from pass_b) so the in-order ACT SEQ never sits
            # on a full DMA request queue ahead of the squares
            pending_loads = [(g * WQG // XG, lambda g=g: load_wq(g))
                             for g in range(2 if n_wq > 1 else 1, n_wq)]
            pending_loads += [(g * WKG // XG, lambda g=g: load_wkv(g))
                             for g in range(1, HC // WKG)]
            pending_loads.sort(key=lambda t: t[0])
            nc.vector.memset(ones_f, 1.0)
            nc.scalar.copy(ones_sb, ones_f)
            nc.scalar.copy(ones_bf, ones_f)
            nc.vector.memset(eps_sb, EPS)
            make_identity(nc, ident_sb)

            # ---- phase 1: fused norm stats + q/k/v projections, one pass
            # over x^T in fp32r ----
            def pass_b(st):
                ss = bass.ts(st, ST)
                cs_sb = tabin.tile([128, ST], F32, tag="cosin", name="cs_sb")
                nc.sync.dma_start(out=cs_sb, in_=cosT[:, ss])
                sn_sb = tabin.tile([128, ST], F32, tag="sinin", name="sn_sb")
                nc.sync.dma_start(out=sn_sb, in_=sinTs[:, ss])
                sq_ps = acc_ps.tile([1, ST], F32, tag="acc", name="sq_ps")
                q_ps = [acc_ps.tile([128, ST], F32, tag="acc", name=f"q_ps{m}")
                        for m in range(qh)]
                k_ps = acc_ps.tile([128, ST], F32, tag="acc", name="k_ps")
                v_ps = acc_ps.tile([128, ST], F32, tag="acc", name="v_ps")
                for hg in range(HC // XG):
                    while (st == 0 and pending_loads
                           and pending_loads[0][0] <= hg + 3):
                        pending_loads.pop(0)[1]()
                    if st == 0 and hg < len(x_pre):
                        x_sb = x_pre[hg]
                    else:
                        x_sb = xin.tile([128, XG, ST], BF16, tag="x",
                                        name="x_sb", bufs=4)
                        nc.sync.dma_start(out=x_sb,
                                          in_=xT_t[:, hg * XG:(hg + 1) * XG,
                                                   ss])
                    for hi in range(XG):
                        hc = hg * XG + hi
                        xs = x_sb[:, hi, :]
                        st_, sp_ = (hc == 0), (hc == HC - 1)
                        x2_sb = probs.tile([128, ST], BF16, tag="p",
                                           name="x2_sb", bufs=6)
                        if hc % 2 == 0:
                            nc.scalar.square(x2_sb, xs)
                        else:
                            nc.gpsimd.tensor_mul(x2_sb, xs, xs)
                        nc.tensor.matmul(sq_ps, ones_bf, x2_sb,
                                         start=st_, stop=sp_)
                        for m in range(qh):
                            nc.tensor.matmul(
                                q_ps[m], wq_sb[:, hc, bass.ts(m, 128)], xs,
                                start=st_, stop=sp_,
                            )
                        nc.tensor.matmul(k_ps, wk_sb[:, hc, :], xs,
                                         start=st_, stop=sp_)
                        nc.tensor.matmul(v_ps, wv_sb[:, hc, :], xs,
                                         start=st_, stop=sp_)
                if st == 0:
                    while pending_loads:
                        pending_loads.pop(0)[1]()
                # r = 1/sqrt(mean + eps); fold into cos/sin tables
                sd_sb = statp.tile([1, ST], F32, tag="stat", name="sd_sb")
                nc.scalar.activation(
                    sd_sb, sq_ps, mybir.ActivationFunctionType.Sqrt,
                    bias=eps_sb, scale=1.0 / h,
                )
                rr_sb = statp.tile([1, ST], F32, tag="stat", name="rr_sb")
                nc.vector.reciprocal(rr_sb, sd_sb)
                R_t = tabp.tile([128, ST], F32, tag="R", name="R_t",
                                bufs=1)
                nc.gpsimd.partition_broadcast(R_t, rr_sb)
                cp_t = tabp.tile([128, ST], F32, tag="cp", name="cp_t",
                                 bufs=1)
                nc.vector.tensor_mul(cp_t, cs_sb, R_t)
                sp_t = tabp.tile([128, ST], F32, tag="sp", name="sp_t",
                                 bufs=1)
                nc.vector.tensor_mul(sp_t, sn_sb, R_t)

                # evacuation: fast ACT copy frees the PSUM bank, then
                # norm+RoPE happens SBUF-side on DVE (in place; the u-halves
                # read the raw values before the cos-multiply overwrites)
                def rope_xform(dst):
                    u_sb = ropep.tile([128, ST], F32, tag="u", name="u_sb",
                                      bufs=1)
                    nc.vector.tensor_mul(
                        u_sb[0:64, :], dst[64:128, :], sp_t[64:128, :])
                    nc.vector.tensor_mul(
                        u_sb[64:128, :], dst[0:64, :], sp_t[0:64, :])
                    nc.vector.tensor_mul(dst, dst, cp_t)
                    nc.vector.tensor_add(dst, dst, u_sb)

                if diag == "no_evac":
                    return
                nc.scalar.copy(kT_sb[:, ss], k_ps)
                vT_sb = ropep.tile([128, ST], F32, tag="vT", name="vT_sb",
                                   bufs=2)
                nc.vector.tensor_copy(vT_sb, v_ps)
                nc.vector.tensor_mul(vT_sb, vT_sb, R_t)
                for jj in range(ST // 128):
                    jc = st * (ST // 128) + jj
                    vt_ps = acc_ps.tile([128, 128], F32, tag="acc",
                                        name="vt_ps")
                    nc.tensor.transpose(vt_ps, vT_sb[:, bass.ts(jj, 128)],
                                        ident_sb)
                    nc.scalar.copy(vnat_sb[:, jc, :], vt_ps)
                rope_xform(kT_sb[:, ss])
                for m in range(qh):
                    dst = ropep.tile([128, ST], F32, tag="t", name="t_sb",
                                     bufs=2)
                    nc.scalar.copy(dst, q_ps[m])
                    rope_xform(dst)
                    nc.scalar.copy(qT_sb[:, m, ss], dst)

            for st in range(NST):
                pass_b(st)

            # ---- phase 2: wo load (sync queue: idle at the transition) ----
            wo_sb = persist.tile([128, qh, h], BF16, tag="bigw")
            if stop_after != "p1":
                for wc in range(h // WOC):
                    nc.sync.dma_start(
                        out=wo_sb[:, :, bass.ts(wc, WOC)],
                        in_=wo_t[:, :, bass.ts(wc, WOC)],
                    )

            # attn^T reuses the wk/wv slots (heads 0-1 / 2-3)
            attnT_h = [
                persist.tile([128, 2, s], BF16, tag="wk", name="attnT01"),
                persist.tile([128, 2, s], BF16, tag="wv", name="attnT23"),
            ]

            def attn_slice(m, sl):
                return attnT_h[m // 2][:, m % 2, sl]

            # ---- phase 3 + 4 interleaved: attention per i-tile (both
            # head pairs), then immediately the o_proj matmuls for that
            # i-range so they fill PE stalls in the next i-tile's attention
            def attn_tile(hp, ti, q_all):
                heads = (2 * hp, 2 * hp + 1)
                iss = bass.ts(ti, ST)
                q_sbs = [q_all[2 * hp], q_all[2 * hp + 1]]
                av_ps = [acc_ps.tile([128, ST], F32, tag="acc",
                                     name=f"av_ps{i}") for i in range(2)]
                z_ps = [acc_ps.tile([1, ST], F32, tag="acc",
                                    name=f"z_ps{i}") for i in range(2)]
                njc = (ti + 1) * (ST // 128)
                for jc in range(njc):
                    st_, sp_ = (jc == 0), (jc == njc - 1)
                    # diagonal chunks: columns left of the 128-wide causal
                    # block (at bcol) are fully masked -> zero them, compute
                    # scores on [dcol, ST) (dcol clamped so the fp32r matmul
                    # stays >=256 wide), exp only on [bcol, ST), and
                    # affine-select the block itself
                    bcol = max(0, jc * 128 - ti * ST)
                    dcol = min(bcol, ST - 256)
                    w = ST - dcol
                    for i in range(2):
                        s_ps = acc_ps.tile([128, w], F32, tag="acc",
                                           name=f"s_ps{i}")
                        nc.tensor.matmul(
                            s_ps, kT_sb[:, bass.ts(jc, 128)],
                            q_sbs[i][:, dcol:],
                            start=True, stop=True,
                        )
                        p_sb = probs.tile([128, ST], BF16, tag="p",
                                          name=f"p_sb{i}", bufs=6)
                        if bcol > 0:
                            nc.vector.memset(p_sb[:, 0:bcol], 0.0)
                        nc.scalar.activation(
                            p_sb[:, bcol:], s_ps[:, bcol - dcol:],
                            mybir.ActivationFunctionType.Exp,
                            scale=scale,
                        )
                        if (jc + 1) * 128 > ti * ST:
                            nc.gpsimd.affine_select(
                                out=p_sb[:, bcol:bcol + 128],
                                in_=p_sb[:, bcol:bcol + 128],
                                pattern=[[1, 128]],
                                compare_op=mybir.AluOpType.is_ge,
                                fill=0.0,
                                base=0,
                                channel_multiplier=-1,
                            )
                        nc.tensor.matmul(av_ps[i], vnat_sb[:, jc, :], p_sb,
                                         start=st_, stop=sp_)
                        nc.tensor.matmul(z_ps[i], ones_bf, p_sb,
                                         start=st_, stop=sp_)
                for i, hh in enumerate(heads):
                    zr_sb = statp.tile([1, ST], F32, tag="stat",
                                       name="zr_sb")
                    nc.vector.reciprocal(zr_sb, z_ps[i])
                    ZR_sb = bcastp.tile([128, ST], F32, tag="bcast",
                                        name="ZR_sb")
                    nc.gpsimd.partition_broadcast(ZR_sb, zr_sb)
                    nc.vector.tensor_mul(attn_slice(hh, iss), av_ps[i],
                                         ZR_sb)

            def o_proj_tile(ti):
                # per i-tile: sc-outer, ht-pair inner; 2 PSUM banks per
                # pair, evacuate into one [128, HTP] tile, single DMA out
                # on the gpsimd SWDGE queue
                for hp in range(NHP):
                    nbl = min(2, NHB - 2 * hp)
                    for sc in range(ti * (ST // 128), (ti + 1) * (ST // 128)):
                        scs = bass.ts(sc, 128)
                        o_ps = [acc_ps.tile([128, 512], F32, tag="acc",
                                            name=f"o_ps{_hh}")
                                for _hh in range(nbl)]
                        for hh in range(nbl):
                            for m in range(qh):
                                nc.tensor.matmul(
                                    o_ps[hh], attn_slice(m, scs),
                                    wo_sb[:, m, bass.ts(2 * hp + hh, 512)],
                                    start=(m == 0), stop=(m == qh - 1),
                                )
                        o_sb = outp.tile([128, nbl, 512], F32, tag="o",
                                         name="o_sb", bufs=4)
                        for hh in range(nbl):
                            if (sc + hp + hh) % 2 == 0:
                                nc.scalar.copy(o_sb[:, hh, :], o_ps[hh])
                            else:
                                nc.vector.tensor_copy(o_sb[:, hh, :], o_ps[hh])
                        if ti == NST - 1 and sc % 4 == 3:
                            engs = [nc.sync, nc.scalar, nc.gpsimd]
                            for hh in range(nbl):
                                engs[(2 * hp + hh) % 3].dma_start(
                                    out=out[scs, bass.ts(2 * hp + hh, 512)],
                                    in_=o_sb[:, hh, :],
                                )
                        else:
                            eng = (nc.sync if (sc + hp) % 2 == 0
                                   else nc.gpsimd)
                            eng.dma_start(
                                out=out[scs,
                                        2 * hp * 512:(2 * hp + nbl) * 512],
                                in_=o_sb,
                            )

            if stop_after not in ("p1", "p2"):
                def attn_full_tile(ti):
                    iss = bass.ts(ti, ST)
                    q_all = [qT_sb[:, hh, iss] for hh in range(qh)]
                    for hp in range(qh // 2):
                        attn_tile(hp, ti, q_all)

                attn_full_tile(0)
                for ti in range(NST):
                    if ti + 1 < NST:
                        attn_full_tile(ti + 1)
                    if stop_after is None:
                        o_proj_tile(ti)

    nc.compile()
    return nc


def make_core_inputs(hidden_states, cos, sin, norm_w, wq, wk, wv, wo,
                     s=S, h=H, qh=QH, n_cores=N_CORES):
    """Host-side sharding + layout preparation. Returns list of in_maps."""
    import ml_dtypes

    dq = qh * HD
    dkv = DKV
    x = np.asarray(hidden_states, dtype=np.float32).reshape(s, h)
    nw = np.asarray(norm_w, dtype=np.float32)
    xT = np.ascontiguousarray(x.T)                      # [h, s]
    cosT = np.ascontiguousarray(np.asarray(cos, np.float32).reshape(s, HD).T)
    sinT = np.ascontiguousarray(np.asarray(sin, np.float32).reshape(s, HD).T)
    # swapped/sign-flipped sin table: rows 0:64 = +sin_half, 64:128 = -sin_half
    sin_half = sinT[0:64]
    sinTs = np.ascontiguousarray(np.concatenate([sinT[64:128], -sin_half], axis=0))
    # fold norm_w into the projection weights
    wq_f = np.asarray(wq, np.float32) * nw[:, None]
    wk_f = np.asarray(wk, np.float32) * nw[:, None]
    wv_f = np.asarray(wv, np.float32) * nw[:, None]
    wo_f = np.asarray(wo, np.float32)

    in_maps = []
    for c in range(n_cores):
        in_maps.append({
            "xT": xT.astype(ml_dtypes.bfloat16),
            "wq": np.ascontiguousarray(
                wq_f[:, c * dq:(c + 1) * dq].astype(ml_dtypes.bfloat16)),
            "wk": np.ascontiguousarray(
                wk_f[:, c * dkv:(c + 1) * dkv].astype(ml_dtypes.bfloat16)),
            "wv": np.ascontiguousarray(
                wv_f[:, c * dkv:(c + 1) * dkv].astype(ml_dtypes.bfloat16)),
            "wo": np.ascontiguousarray(wo_f[c * dq:(c + 1) * dq, :]
                                       .astype(ml_dtypes.bfloat16)),
            "cosT": cosT,
            "sinTs": sinTs,
        })
    return in_maps


_NC_CACHE = {}


def kernel(hidden_states, cos, sin, norm_w, wq, wk, wv, wo):
    from concourse.bass_utils import run_bass_kernel_spmd

    if "nc" not in _NC_CACHE:
        _NC_CACHE["nc"] = build_bass()
    nc = _NC_CACHE["nc"]
    in_maps = make_core_inputs(hidden_states, cos, sin, norm_w, wq, wk, wv, wo)
    res = run_bass_kernel_spmd(nc, in_maps, core_ids=list(range(N_CORES)))
    partials = [m["out"] for m in res.results]
    out = np.asarray(hidden_states, np.float32).reshape(S, H).copy()
    for p in partials:
        out += p
    return out.reshape(B, S, H)


# revision 54
# speedup vs baseline: 1.2629x; 1.0872x over previous
"""Mixtral attention layer (B=1, S=2048, H=4096, NH=32, NKV=8, HD=128) on 8
Trainium2 NeuronCores, tensor-parallel over heads.

Sharding: core c owns 4 query heads + 1 KV head (column-shard of wq/wk/wv,
row-shard of wo).  Each core computes a full [S, H] partial of the o_proj
output; the host sums the 8 partials and adds the residual (the gather of a
row-parallel matmul).

Per-core pipeline (projection/attention matmuls in float32r = fp22-truncated
fp32, full PE rate at N>=256):
  Phase 1 (fused stats + projections): x^T streamed once in fp32r (2-chunk
    group DMAs); per H-chunk, ACT squares x into bf16 and a ones-vector
    matmul accumulates sum(x^2) over H in PSUM alongside the 6 projection
    matmuls (4 q heads + k + v).  r = 1/sqrt(mean+eps) is partition-broadcast
    and folded into full-width RoPE cos/sin tables; PSUM evacuation applies
    norm + RoPE on DVE.  norm_w is folded into the weights on the host.
    Weights are loaded in chunks (wq x4, wk/wv x2) so the first projection
    matmuls start as soon as the first chunks land.
  Attention: per head-pair sweep (both heads share this core's single KV
    head - GQA), causal flash-style: scores^T = k^T.T @ q^T chunkwise
    (narrowed to the valid column range on diagonal chunks), exp on ACT
    (PSUM->SBUF), causal mask via affine_select on the 128-wide diagonal
    block only, unnormalized AV + ones-matmul row-sum Z accumulate in PSUM;
    1/Z applied at AV evacuation into SBUF-resident attn^T.
  o_proj: per i-tile, attn^T @ wo chunked ht-pair-outer; wo is DMA'd in 4
    column chunks so the first matmuls don't wait for the full 8MB load.
    Output DMAs batched per (sc, ht-pair) and issued on the gpsimd SWDGE
    queue to keep them off the HWDGE descriptor path.

q^T is spilled to internal DRAM between phases (SBUF pressure, via the
Activation HWDGE queue); attn^T reuses the wk/wv SBUF slots after the
projections retire.
"""

import math

import numpy as np

import concourse.bass as bass
import concourse.tile as tile
from concourse import bacc, mybir
from concourse.masks import make_identity

F32 = mybir.dt.float32
F32R = mybir.dt.float32r
BF16 = mybir.dt.bfloat16

# Full problem dims
B, S, H, NH, NKV, HD = 1, 2048, 4096, 32, 8, 128
EPS = 1e-5
N_CORES = 8
QH = NH // N_CORES          # query heads per core = 4
DQ = QH * HD                # q columns per core = 512
DKV = (NKV // N_CORES) * HD  # kv columns per core = 128


def build_bass(s=S, h=H, qh=QH, stop_after=None, diag=None):
    """Build the single-core Bass module (same NEFF on all 8 cores)."""
    ST = 512 if s >= 512 else s       # s-tile width (proj + attention i-tiles)
    NST = s // ST                     # number of s-tiles
    HC = h // 128                     # H contraction chunks
    NJ = s // 128                     # j chunks (keys)
    dq = qh * HD
    scale = 1.0 / math.sqrt(HD)
    XG = 2                            # x chunks per DMA group
    WQG = max(1, HC // 8)             # wq chunks per DMA group
    WKG = max(1, HC // 2)             # wk/wv chunks per DMA group
    NHB = h // 512                    # o_proj 512-col blocks
    NHP = (NHB + 1) // 2              # block pairs per sc chunk
    HTP = 1024                        # o_proj ht-pair width (2 PSUM banks)
    WOC = 512                         # wo load chunk width

    nc = bacc.Bacc(None, target_bir_lowering=False)

    xT = nc.dram_tensor("xT", [h, s], BF16, kind="ExternalInput")
    wq = nc.dram_tensor("wq", [h, dq], BF16, kind="ExternalInput")
    wk = nc.dram_tensor("wk", [h, DKV], BF16, kind="ExternalInput")
    wv = nc.dram_tensor("wv", [h, DKV], BF16, kind="ExternalInput")
    wo = nc.dram_tensor("wo", [dq, h], BF16, kind="ExternalInput")
    cosT = nc.dram_tensor("cosT", [HD, s], F32, kind="ExternalInput")
    sinTs = nc.dram_tensor("sinTs", [HD, s], F32, kind="ExternalInput")
    out = nc.dram_tensor("out", [s, h], F32, kind="ExternalOutput")

    xT_t = xT.rearrange("(ho hi) s -> hi ho s", hi=128)
    wq_t = wq.rearrange("(ho hi) d -> hi ho d", hi=128)
    wk_t = wk.rearrange("(ho hi) d -> hi ho d", hi=128)
    wv_t = wv.rearrange("(ho hi) d -> hi ho d", hi=128)
    wo_t = wo.rearrange("(do di) h -> di do h", di=128)

    with tile.TileContext(nc) as tc:
        with (
            tc.tile_pool(name="persist", bufs=1) as persist,
            tc.tile_pool(name="xin", bufs=4) as xin,
                        tc.tile_pool(name="rope", bufs=3) as ropep,
            tc.tile_pool(name="statp", bufs=2) as statp,
            tc.tile_pool(name="tabp", bufs=2) as tabp,
            tc.tile_pool(name="tabin", bufs=1) as tabin,
            tc.tile_pool(name="outp", bufs=4) as outp,
            tc.tile_pool(name="bcastp", bufs=1) as bcastp,
            tc.tile_pool(name="probs", bufs=6) as probs,
            tc.tile_pool(name="acc_ps", bufs=8, space="PSUM") as acc_ps,
        ):
            # ---- persistent SBUF tensors ----
            # Slot reuse chains (same tag, sequential lifetimes):
            #   wq (8MB) -> wo (8MB)         tag "bigw"
            #   wk (2MB) -> attnT heads 0-1  tag "wk"
            #   wv (2MB) -> attnT heads 2-3  tag "wv"
            #   cos (1MB) -> v natural (1MB) tag "cosvnat"
            wq_sb = persist.tile([128, HC, dq], BF16, tag="bigw")
            wk_sb = persist.tile([128, HC, DKV], BF16, tag="wk")
            wv_sb = persist.tile([128, HC, DKV], BF16, tag="wv")
            ones_f = persist.tile([128, 1], F32, tag="ones_f")
            ones_sb = persist.tile([128, 1], F32R, tag="ones")
            ones_bf = persist.tile([128, 1], BF16, tag="ones_bf")
            eps_sb = persist.tile([1, 1], F32, tag="eps")
            ident_sb = persist.tile([128, 128], F32, tag="ident")
            kT_sb = persist.tile([128, s], BF16, tag="kT")
            vnat_sb = persist.tile([128, NJ, 128], BF16, tag="vnat")
            # q^T stays SBUF-resident in bf16 (scores run in bf16)
            qT_sb = persist.tile([128, qh, s], BF16, tag="qT")

            # chunked weight loads: the first two x groups of tile 0 are
            # requested ahead of the weights so the DMA FIFO serves them
            # first; wk/wv before wq (k/v matmuls are ordered first per
            # chunk), wq in small chunks so early q matmuls start fast
            x_pre = []
            for hg in range(min(3, HC // XG)):
                x_sb = xin.tile([128, XG, ST], BF16, tag="x",
                                name=f"x_pre{hg}", bufs=4)
                nc.sync.dma_start(out=x_sb,
                                  in_=xT_t[:, hg * XG:(hg + 1) * XG,
                                           bass.ts(0, ST)])
                x_pre.append(x_sb)
            def load_wq(g):
                nc.sync.dma_start(out=wq_sb[:, g * WQG:(g + 1) * WQG, :],
                                  in_=wq_t[:, g * WQG:(g + 1) * WQG, :])

            def load_wkv(g):
                nc.sync.dma_start(out=wk_sb[:, g * WKG:(g + 1) * WKG, :],
                                  in_=wk_t[:, g * WKG:(g + 1) * WKG, :])
                nc.sync.dma_start(out=wv_sb[:, g * WKG:(g + 1) * WKG, :],
                                  in_=wv_t[:, g * WKG:(g + 1) * WKG, :])

            n_wq = HC // WQG
            load_wq(0)
            load_wkv(0)
            if n_wq > 1:
                load_wq(1)
            # remaining weight-chunk dispatches are spread through tile 0's
            # loop (emitted from pass_b) so the in-order ACT SEQ never sits
            # on a full DMA request queue ahead of the squares
            pending_loads = [(g * WQG // XG, lambda g=g: load_wq(g))
                             for g in range(2 if n_wq > 1 else 1, n_wq)]
            pending_loads += [(g * WKG // XG, lambda g=g: load_wkv(g))
                             for g in range(1, HC // WKG)]
            pending_loads.sort(key=lambda t: t[0])
            nc.vector.memset(ones_f, 1.0)
            nc.scalar.copy(ones_sb, ones_f)
            nc.scalar.copy(ones_bf, ones_f)
            nc.vector.memset(eps_sb, EPS)
            make_identity(nc, ident_sb)

            # ---- phase 1: fused norm stats + q/k/v projections, one pass
            # over x^T in fp32r ----
            deferred_tp = []

            def pass_b(st):
                ss = bass.ts(st, ST)
                cs_sb = tabin.tile([128, ST], F32, tag="cosin", name="cs_sb")
                sn_sb = tabin.tile([128, ST], F32, tag="sinin", name="sn_sb")

                def load_tabs():
                    nc.sync.dma_start(out=cs_sb, in_=cosT[:, ss])
                    nc.sync.dma_start(out=sn_sb, in_=sinTs[:, ss])

                if st == 0:
                    pending_loads.append((max(0, HC // XG - 6), load_tabs))
                    pending_loads.sort(key=lambda t: t[0])
                else:
                    load_tabs()
                sq_ps = acc_ps.tile([1, ST], F32, tag="acc", name="sq_ps")
                q_ps = [acc_ps.tile([128, ST], F32, tag="acc", name=f"q_ps{m}")
                        for m in range(qh)]
                k_ps = acc_ps.tile([128, ST], F32, tag="acc", name="k_ps")
                v_ps = acc_ps.tile([128, ST], F32, tag="acc", name="v_ps")
                for hg in range(HC // XG):
                    while (st == 0 and pending_loads
                           and pending_loads[0][0] <= hg + 3):
                        pending_loads.pop(0)[1]()
                    if hg == min(2, HC // XG - 1) and deferred_tp:
                        deferred_tp.pop(0)()
                    if st == 0 and hg < len(x_pre):
                        x_sb = x_pre[hg]
                    else:
                        x_sb = xin.tile([128, XG, ST], BF16, tag="x",
                                        name="x_sb", bufs=4)
                        nc.sync.dma_start(out=x_sb,
                                          in_=xT_t[:, hg * XG:(hg + 1) * XG,
                                                   ss])
                    for hi in range(XG):
                        hc = hg * XG + hi
                        xs = x_sb[:, hi, :]
                        st_, sp_ = (hc == 0), (hc == HC - 1)
                        x2_sb = probs.tile([128, ST], BF16, tag="p",
                                           name="x2_sb", bufs=6)
                        if hc % 2 == 0:
                            nc.scalar.square(x2_sb, xs)
                            x2_prev = x2_sb
                        else:
                            nc.gpsimd.tensor_mul(x2_sb, xs, xs)
                            nc.vector.tensor_add(x2_sb, x2_sb, x2_prev)
                            nc.tensor.matmul(sq_ps, ones_bf, x2_sb,
                                             start=(hc == 1),
                                             stop=(hc == HC - 1))
                        for m in range(qh):
                            nc.tensor.matmul(
                                q_ps[m], wq_sb[:, hc, bass.ts(m, 128)], xs,
                                start=st_, stop=sp_,
                            )
                        nc.tensor.matmul(k_ps, wk_sb[:, hc, :], xs,
                                         start=st_, stop=sp_)
                        nc.tensor.matmul(v_ps, wv_sb[:, hc, :], xs,
                                         start=st_, stop=sp_)
                if st == 0:
                    while pending_loads:
                        pending_loads.pop(0)[1]()
                # r = 1/sqrt(mean + eps); fold into cos/sin tables
                sd_sb = statp.tile([1, ST], F32, tag="stat", name="sd_sb")
                nc.scalar.activation(
                    sd_sb, sq_ps, mybir.ActivationFunctionType.Sqrt,
                    bias=eps_sb, scale=1.0 / h,
                )
                rr_sb = statp.tile([1, ST], F32, tag="stat", name="rr_sb")
                nc.vector.reciprocal(rr_sb, sd_sb)
                R_t = tabp.tile([128, ST], F32, tag="R", name="R_t",
                                bufs=1)
                nc.gpsimd.partition_broadcast(R_t, rr_sb)
                cp_t = tabp.tile([128, ST], F32, tag="cp", name="cp_t",
                                 bufs=1)
                nc.vector.tensor_mul(cp_t, cs_sb, R_t)
                sp_t = tabp.tile([128, ST], F32, tag="sp", name="sp_t",
                                 bufs=1)
                nc.vector.tensor_mul(sp_t, sn_sb, R_t)

                # evacuation: fast ACT copy frees the PSUM bank, then
                # norm+RoPE happens SBUF-side on DVE (in place; the u-halves
                # read the raw values before the cos-multiply overwrites)
                def rope_xform(dst):
                    u_sb = ropep.tile([128, ST], F32, tag="u", name="u_sb",
                                      bufs=1)
                    nc.vector.tensor_mul(
                        u_sb[0:64, :], dst[64:128, :], sp_t[64:128, :])
                    nc.vector.tensor_mul(
                        u_sb[64:128, :], dst[0:64, :], sp_t[0:64, :])
                    nc.vector.tensor_mul(dst, dst, cp_t)
                    nc.vector.tensor_add(dst, dst, u_sb)

                if diag == "no_evac":
                    return
                nc.scalar.copy(kT_sb[:, ss], k_ps)
                vT_sb = ropep.tile([128, ST], F32, tag="vT", name="vT_sb",
                                   bufs=2)
                nc.vector.tensor_copy(vT_sb, v_ps)
                nc.vector.tensor_mul(vT_sb, vT_sb, R_t)

                def do_transposes(st=st, vT_sb=vT_sb):
                    for jj in range(ST // 128):
                        jc = st * (ST // 128) + jj
                        vt_ps = acc_ps.tile([128, 128], F32, tag="acc",
                                            name="vt_ps")
                        nc.tensor.transpose(vt_ps,
                                            vT_sb[:, bass.ts(jj, 128)],
                                            ident_sb)
                        nc.scalar.copy(vnat_sb[:, jc, :], vt_ps)

                if st + 1 < NST:
                    deferred_tp.append(do_transposes)
                else:
                    do_transposes()
                rope_xform(kT_sb[:, ss])
                q_copy = [nc.scalar.copy, nc.gpsimd.tensor_copy,
                          nc.scalar.copy, nc.gpsimd.tensor_copy]
                for m in range(qh):
                    dst = ropep.tile([128, ST], F32, tag="t", name="t_sb",
                                     bufs=2)
                    q_copy[m](dst, q_ps[m])
                    rope_xform(dst)
                    nc.scalar.copy(qT_sb[:, m, ss], dst)

            for st in range(NST):
                pass_b(st)

            # ---- phase 2: wo load (sync queue: idle at the transition) ----
            wo_sb = persist.tile([128, qh, h], BF16, tag="bigw")
            if stop_after != "p1":
                for wc in range(h // WOC):
                    nc.sync.dma_start(
                        out=wo_sb[:, :, bass.ts(wc, WOC)],
                        in_=wo_t[:, :, bass.ts(wc, WOC)],
                    )

            # attn^T reuses the wk/wv slots (heads 0-1 / 2-3)
            attnT_h = [
                persist.tile([128, 2, s], BF16, tag="wk", name="attnT01"),
                persist.tile([128, 2, s], BF16, tag="wv", name="attnT23"),
            ]

            def attn_slice(m, sl):
                return attnT_h[m // 2][:, m % 2, sl]

            # ---- phase 3 + 4 interleaved: attention per i-tile (both
            # head pairs), then immediately the o_proj matmuls for that
            # i-range so they fill PE stalls in the next i-tile's attention
            def attn_tile(hp, ti, q_all):
                heads = (2 * hp, 2 * hp + 1)
                iss = bass.ts(ti, ST)
                q_sbs = [q_all[2 * hp], q_all[2 * hp + 1]]
                av_ps = [acc_ps.tile([128, ST], F32, tag="acc",
                                     name=f"av_ps{i}") for i in range(2)]
                z_ps = [acc_ps.tile([1, ST], F32, tag="acc",
                                    name=f"z_ps{i}") for i in range(2)]
                njc = (ti + 1) * (ST // 128)
                for jc in range(njc):
                    st_, sp_ = (jc == 0), (jc == njc - 1)
                    # diagonal chunks: columns left of the 128-wide causal
                    # block (at bcol) are fully masked -> zero them, compute
                    # scores on [dcol, ST) (dcol clamped so the fp32r matmul
                    # stays >=256 wide), exp only on [bcol, ST), and
                    # affine-select the block itself
                    # diagonal chunks: everything (scores, exp, AV, Z)
                    # narrows to the valid columns [bcol, ST); bf16 matmuls
                    # run at full rate at any width.  PSUM accumulation into
                    # av/z starts full-width at jc==0, later chunks
                    # accumulate into the [bcol:] sub-range only (the
                    # skipped columns would add zero).
                    bcol = max(0, jc * 128 - ti * ST)
                    w = ST - bcol
                    for i in range(2):
                        s_ps = acc_ps.tile([128, w], F32, tag="acc",
                                           name=f"s_ps{i}")
                        nc.tensor.matmul(
                            s_ps, kT_sb[:, bass.ts(jc, 128)],
                            q_sbs[i][:, bcol:],
                            start=True, stop=True,
                        )
                        p_sb = probs.tile([128, w], BF16, tag="p",
                                          name=f"p_sb{i}", bufs=6)
                        nc.scalar.activation(
                            p_sb, s_ps,
                            mybir.ActivationFunctionType.Exp,
                            scale=scale,
                        )
                        if (jc + 1) * 128 > ti * ST:
                            nc.gpsimd.affine_select(
                                out=p_sb[:, 0:128],
                                in_=p_sb[:, 0:128],
                                pattern=[[1, 128]],
                                compare_op=mybir.AluOpType.is_ge,
                                fill=0.0,
                                base=0,
                                channel_multiplier=-1,
                            )
                        nc.tensor.matmul(av_ps[i][:, bcol:],
                                         vnat_sb[:, jc, :], p_sb,
                                         start=st_, stop=sp_)
                        nc.tensor.matmul(z_ps[i][:, bcol:], ones_bf, p_sb,
                                         start=st_, stop=sp_)
                for i, hh in enumerate(heads):
                    zr_sb = statp.tile([1, ST], F32, tag="stat",
                                       name="zr_sb")
                    nc.vector.reciprocal(zr_sb, z_ps[i])
                    ZR_sb = bcastp.tile([128, ST], F32, tag="bcast",
                                        name="ZR_sb")
                    nc.gpsimd.partition_broadcast(ZR_sb, zr_sb)
                    nc.vector.tensor_mul(attn_slice(hh, iss), av_ps[i],
                                         ZR_sb)

            def o_proj_tile(ti):
                # per i-tile: sc-outer, ht-pair inner; 2 PSUM banks per
                # pair, evacuate into one [128, HTP] tile, single DMA out
                # on the gpsimd SWDGE queue
                for hp in range(NHP):
                    nbl = min(2, NHB - 2 * hp)
                    for sc in range(ti * (ST // 128), (ti + 1) * (ST // 128)):
                        scs = bass.ts(sc, 128)
                        o_ps = [acc_ps.tile([128, 512], F32, tag="acc",
                                            name=f"o_ps{_hh}")
                                for _hh in range(nbl)]
                        for hh in range(nbl):
                            for m in range(qh):
                                nc.tensor.matmul(
                                    o_ps[hh], attn_slice(m, scs),
                                    wo_sb[:, m, bass.ts(2 * hp + hh, 512)],
                                    start=(m == 0), stop=(m == qh - 1),
                                )
                        o_sb = outp.tile([128, nbl, 512], F32, tag="o",
                                         name="o_sb", bufs=4)
                        for hh in range(nbl):
                            r = (sc + hp + hh) % 3
                            if r == 0:
                                nc.scalar.copy(o_sb[:, hh, :], o_ps[hh])
                            elif r == 1:
                                nc.vector.tensor_copy(o_sb[:, hh, :], o_ps[hh])
                            else:
                                nc.gpsimd.tensor_copy(o_sb[:, hh, :], o_ps[hh])
                        if ti == NST - 1 and sc % 4 == 3:
                            engs = [nc.sync, nc.scalar, nc.gpsimd]
                            for hh in range(nbl):
                                engs[(2 * hp + hh) % 3].dma_start(
                                    out=out[scs, bass.ts(2 * hp + hh, 512)],
                                    in_=o_sb[:, hh, :],
                                )
                        else:
                            eng = (nc.sync if (sc + hp) % 2 == 0
                                   else nc.gpsimd)
                            eng.dma_start(
                                out=out[scs,
                                        2 * hp * 512:(2 * hp + nbl) * 512],
                                in_=o_sb,
                            )

            if stop_after not in ("p1", "p2"):
                def attn_full_tile(ti):
                    iss = bass.ts(ti, ST)
                    q_all = [qT_sb[:, hh, iss] for hh in range(qh)]
                    for hp in range(qh // 2):
                        attn_tile(hp, ti, q_all)

                attn_full_tile(0)
                for ti in range(NST):
                    if ti + 1 < NST:
                        attn_full_tile(ti + 1)
                    if stop_after is None:
                        o_proj_tile(ti)

    nc.compile()
    return nc


def make_core_inputs(hidden_states, cos, sin, norm_w, wq, wk, wv, wo,
                     s=S, h=H, qh=QH, n_cores=N_CORES):
    """Host-side sharding + layout preparation. Returns list of in_maps."""
    import ml_dtypes

    dq = qh * HD
    dkv = DKV
    x = np.asarray(hidden_states, dtype=np.float32).reshape(s, h)
    nw = np.asarray(norm_w, dtype=np.float32)
    xT = np.ascontiguousarray(x.T)                      # [h, s]
    cosT = np.ascontiguousarray(np.asarray(cos, np.float32).reshape(s, HD).T)
    sinT = np.ascontiguousarray(np.asarray(sin, np.float32).reshape(s, HD).T)
    # swapped/sign-flipped sin table: rows 0:64 = +sin_half, 64:128 = -sin_half
    sin_half = sinT[0:64]
    sinTs = np.ascontiguousarray(np.concatenate([sinT[64:128], -sin_half], axis=0))
    # fold norm_w into the projection weights
    wq_f = np.asarray(wq, np.float32) * nw[:, None]
    wk_f = np.asarray(wk, np.float32) * nw[:, None]
    wv_f = np.asarray(wv, np.float32) * nw[:, None]
    wo_f = np.asarray(wo, np.float32)

    in_maps = []
    for c in range(n_cores):
        in_maps.append({
            "xT": xT.astype(ml_dtypes.bfloat16),
            "wq": np.ascontiguousarray(
                wq_f[:, c * dq:(c + 1) * dq].astype(ml_dtypes.bfloat16)),
            "wk": np.ascontiguousarray(
                wk_f[:, c * dkv:(c + 1) * dkv].astype(ml_dtypes.bfloat16)),
            "wv": np.ascontiguousarray(
                wv_f[:, c * dkv:(c + 1) * dkv].astype(ml_dtypes.bfloat16)),
            "wo": np.ascontiguousarray(wo_f[c * dq:(c + 1) * dq, :]
                                       .astype(ml_dtypes.bfloat16)),
            "cosT": cosT,
            "sinTs": sinTs,
        })
    return in_maps


_NC_CACHE = {}


def kernel(hidden_states, cos, sin, norm_w, wq, wk, wv, wo):
    from concourse.bass_utils import run_bass_kernel_spmd

    if "nc" not in _NC_CACHE:
        _NC_CACHE["nc"] = build_bass()
    nc = _NC_CACHE["nc"]
    in_maps = make_core_inputs(hidden_states, cos, sin, norm_w, wq, wk, wv, wo)
    res = run_bass_kernel_spmd(nc, in_maps, core_ids=list(range(N_CORES)))
    partials = [m["out"] for m in res.results]
    out = np.asarray(hidden_states, np.float32).reshape(S, H).copy()
    for p in partials:
        out += p
    return out.reshape(B, S, H)


# revision 79
# speedup vs baseline: 1.3614x; 1.0780x over previous
"""Mixtral attention layer (B=1, S=2048, H=4096, NH=32, NKV=8, HD=128) on 8
Trainium2 NeuronCores, tensor-parallel over heads.

Sharding: core c owns 4 query heads + 1 KV head (column-shard of wq/wk/wv,
row-shard of wo).  Each core computes a full [S, H] partial of the o_proj
output; the host sums the 8 partials and adds the residual (the gather of a
row-parallel matmul).

Per-core pipeline (bf16 weights/activations on the PE, fp32 PSUM
accumulation everywhere; sized against the TimelineSim cost model):
  Phase 1 (fused stats + projections): x^T streamed once in bf16 (2-chunk
    group DMAs on the SP queue, weights interleaved in need order); per
    H-chunk the 6 projection matmuls (4 q heads + k + v) accumulate in
    PSUM while x^2 is squared (ACT/DVE) and pair-summed so a ones-vector
    matmul reduces sum(x^2) over H every second chunk.  r =
    1/sqrt(mean+eps) folds into per-tile RoPE cos/sin tables; PSUM
    evacuation does norm+RoPE on DVE with PSUM-freeing copies spread over
    ACT/Pool.  v is transposed to natural layout per tile (transposes
    deferred into the next tile's stream so the stats chain never blocks
    the in-order PE queue).  norm_w is folded into the weights host-side.
  Attention: per head-pair sweep (GQA: both heads share the core's single
    KV head), causal flash-style: scores^T = k^T.T @ q^T per 128-key
    chunk in bf16 (diagonal chunks narrowed to the valid column range),
    exp on ACT, causal mask via affine_select on the 128-wide diagonal
    block, unnormalized AV + ones-matmul row-sum Z accumulate in PSUM;
    1/Z applied at AV evacuation into per-tile SBUF attn^T tiles.
    Attention runs one i-tile ahead of o_proj.
  o_proj: per i-tile, attn^T @ wo (bf16, own SBUF slot, chunk-loaded at
    the phase boundary); [128, 1024] output tiles DMA out alternating the
    SP HWDGE and gpsimd SWDGE queues, the last s-chunk split 512-wide
    across three queues to shorten the drain tail.

q^T/k^T stay SBUF-resident in bf16 (one tile per s-tile); attn^T is
per-i-tile.  All timing tuned against TimelineSim (per-DMA HWDGE cost,
serial DMA_ENGINES device, in-order engine queues, PE p-state ramp).
"""

import math

import numpy as np

import concourse.bass as bass
import concourse.tile as tile
from concourse import bacc, mybir
from concourse.masks import make_identity

F32 = mybir.dt.float32
F32R = mybir.dt.float32r
BF16 = mybir.dt.bfloat16

# Full problem dims
B, S, H, NH, NKV, HD = 1, 2048, 4096, 32, 8, 128
EPS = 1e-5
N_CORES = 8
QH = NH // N_CORES          # query heads per core = 4
DQ = QH * HD                # q columns per core = 512
DKV = (NKV // N_CORES) * HD  # kv columns per core = 128


def build_bass(s=S, h=H, qh=QH, stop_after=None, diag=None):
    """Build the single-core Bass module (same NEFF on all 8 cores)."""
    ST = 512 if s >= 512 else s       # s-tile width (proj + attention i-tiles)
    NST = s // ST                     # number of s-tiles
    HC = h // 128                     # H contraction chunks
    NJ = s // 128                     # j chunks (keys)
    dq = qh * HD
    scale = 1.0 / math.sqrt(HD)
    XG = 2                            # x chunks per DMA group
    WQG = max(1, HC // 8)             # wq chunks per DMA group
    WKG = max(1, HC // 2)             # wk/wv chunks per DMA group
    NHB = h // 512                    # o_proj 512-col blocks
    NHP = (NHB + 1) // 2              # block pairs per sc chunk
    HTP = 1024                        # o_proj ht-pair width (2 PSUM banks)
    WOC = 512                         # wo load chunk width

    nc = bacc.Bacc(None, target_bir_lowering=False)

    xT = nc.dram_tensor("xT", [h, s], BF16, kind="ExternalInput")
    wq = nc.dram_tensor("wq", [h, dq], BF16, kind="ExternalInput")
    wk = nc.dram_tensor("wk", [h, DKV], BF16, kind="ExternalInput")
    wv = nc.dram_tensor("wv", [h, DKV], BF16, kind="ExternalInput")
    wo = nc.dram_tensor("wo", [dq, h], BF16, kind="ExternalInput")
    cosT = nc.dram_tensor("cosT", [HD, s], F32, kind="ExternalInput")
    sinTs = nc.dram_tensor("sinTs", [HD, s], F32, kind="ExternalInput")
    out = nc.dram_tensor("out", [s, h], F32, kind="ExternalOutput")

    xT_t = xT.rearrange("(ho hi) s -> hi ho s", hi=128)
    wq_t = wq.rearrange("(ho hi) d -> hi ho d", hi=128)
    wk_t = wk.rearrange("(ho hi) d -> hi ho d", hi=128)
    wv_t = wv.rearrange("(ho hi) d -> hi ho d", hi=128)
    wo_t = wo.rearrange("(do di) h -> di do h", di=128)

    with tile.TileContext(nc) as tc:
        with (
            tc.tile_pool(name="persist", bufs=1) as persist,
            tc.tile_pool(name="xin", bufs=4) as xin,
                        tc.tile_pool(name="rope", bufs=3) as ropep,
            tc.tile_pool(name="statp", bufs=2) as statp,
            tc.tile_pool(name="tabp", bufs=2) as tabp,
            tc.tile_pool(name="tabin", bufs=1) as tabin,
            tc.tile_pool(name="outp", bufs=4) as outp,
            tc.tile_pool(name="bcastp", bufs=1) as bcastp,
            tc.tile_pool(name="probs", bufs=6) as probs,
            tc.tile_pool(name="acc_ps", bufs=8, space="PSUM") as acc_ps,
        ):
            # ---- persistent SBUF tensors ----
            # Slot reuse chains (same tag, sequential lifetimes):
            #   wq (8MB) -> wo (8MB)         tag "bigw"
            #   wk (2MB) -> attnT heads 0-1  tag "wk"
            #   wv (2MB) -> attnT heads 2-3  tag "wv"
            #   cos (1MB) -> v natural (1MB) tag "cosvnat"
            wq_sb = persist.tile([128, HC, dq], BF16, tag="bigw")
            wo_sb = persist.tile([128, qh, h], BF16, tag="wo")
            wk_sb = persist.tile([128, HC, DKV], BF16, tag="wk")
            wv_sb = persist.tile([128, HC, DKV], BF16, tag="wv")
            ones_f = persist.tile([128, 1], F32, tag="ones_f")
            ones_bf = persist.tile([128, 1], BF16, tag="ones_bf")
            eps_sb = persist.tile([1, 1], F32, tag="eps")
            ident_sb = persist.tile([128, 128], F32, tag="ident")
            kT_t = [persist.tile([128, ST], BF16, tag=f"kT{_t}",
                                 name=f"kT_t{_t}") for _t in range(NST)]
            vnat_sb = persist.tile([128, NJ, 128], BF16, tag="vnat")
            # q^T stays SBUF-resident in bf16 (scores run in bf16),
            # one tile per s-tile for precise dependency tracking
            qT_t = [persist.tile([128, qh, ST], BF16, tag=f"qT{_t}",
                                 name=f"qT_t{_t}") for _t in range(NST)]

            # chunked weight loads: the first two x groups of tile 0 are
            # requested ahead of the weights so the DMA FIFO serves them
            # first; wk/wv before wq (k/v matmuls are ordered first per
            # chunk), wq in small chunks so early q matmuls start fast
            def load_wq(g):
                nc.sync.dma_start(out=wq_sb[:, g * WQG:(g + 1) * WQG, :],
                                  in_=wq_t[:, g * WQG:(g + 1) * WQG, :])

            def load_wkv(g):
                nc.sync.dma_start(out=wk_sb[:, g * WKG:(g + 1) * WKG, :],
                                  in_=wk_t[:, g * WKG:(g + 1) * WKG, :])
                nc.sync.dma_start(out=wv_sb[:, g * WKG:(g + 1) * WKG, :],
                                  in_=wv_t[:, g * WKG:(g + 1) * WKG, :])

            n_wq = HC // WQG
            load_wq(0)
            x_pre = []
            for hg in range(min(3, HC // XG)):
                x_sb = xin.tile([128, XG, ST], BF16, tag="x",
                                name=f"x_pre{hg}", bufs=4)
                nc.sync.dma_start(out=x_sb,
                                  in_=xT_t[:, hg * XG:(hg + 1) * XG,
                                           bass.ts(0, ST)])
                x_pre.append(x_sb)
            load_wkv(0)
            if n_wq > 1:
                load_wq(1)
            # remaining weight-chunk dispatches are spread through tile 0's
            # loop (emitted from pass_b) so the in-order ACT SEQ never sits
            # on a full DMA request queue ahead of the squares
            pending_loads = [(g * WQG // XG, lambda g=g: load_wq(g))
                             for g in range(2 if n_wq > 1 else 1, n_wq)]
            pending_loads += [(g * WKG // XG, lambda g=g: load_wkv(g))
                             for g in range(1, HC // WKG)]
            pending_loads.sort(key=lambda t: t[0])
            nc.vector.memset(ones_f, 1.0)
            nc.vector.tensor_copy(ones_bf, ones_f)
            nc.vector.memset(eps_sb, EPS)
            make_identity(nc, ident_sb)


            # ---- phase 1: fused norm stats + q/k/v projections, one pass
            # over x^T in fp32r ----
            deferred_tp = []

            def pass_b(st):
                ss = bass.ts(st, ST)
                cs_sb = tabin.tile([128, ST], F32, tag="cosin", name="cs_sb")
                sn_sb = tabin.tile([128, ST], F32, tag="sinin", name="sn_sb")

                def load_tabs():
                    nc.sync.dma_start(out=cs_sb, in_=cosT[:, ss])
                    nc.sync.dma_start(out=sn_sb, in_=sinTs[:, ss])

                if st == 0:
                    pending_loads.append((max(0, HC // XG - 6), load_tabs))
                    pending_loads.sort(key=lambda t: t[0])
                else:
                    load_tabs()
                sq_ps = acc_ps.tile([1, ST], F32, tag="acc", name="sq_ps")
                q_ps = [acc_ps.tile([128, ST], F32, tag="acc", name=f"q_ps{m}")
                        for m in range(qh)]
                k_ps = acc_ps.tile([128, ST], F32, tag="acc", name="k_ps")
                v_ps = acc_ps.tile([128, ST], F32, tag="acc", name="v_ps")
                for hg in range(HC // XG):
                    while (st == 0 and pending_loads
                           and pending_loads[0][0] <= hg + 3):
                        pending_loads.pop(0)[1]()
                    if hg == min(2, HC // XG - 1) and deferred_tp:
                        deferred_tp.pop(0)()
                    if st == 0 and hg < len(x_pre):
                        x_sb = x_pre[hg]
                    else:
                        x_sb = xin.tile([128, XG, ST], BF16, tag="x",
                                        name="x_sb", bufs=4)
                        nc.sync.dma_start(out=x_sb,
                                          in_=xT_t[:, hg * XG:(hg + 1) * XG,
                                                   ss])
                    for hi in range(XG):
                        hc = hg * XG + hi
                        xs = x_sb[:, hi, :]
                        st_, sp_ = (hc == 0), (hc == HC - 1)
                        x2_sb = probs.tile([128, ST], BF16, tag="p",
                                           name="x2_sb", bufs=6)
                        if hc % 2 == 0:
                            nc.scalar.square(x2_sb, xs)
                            x2_prev = x2_sb
                        else:
                            nc.vector.tensor_mul(x2_sb, xs, xs)
                            nc.vector.tensor_add(x2_sb, x2_sb, x2_prev)
                            nc.tensor.matmul(sq_ps, ones_bf, x2_sb,
                                             start=(hc == 1),
                                             stop=(hc == HC - 1))
                        for m in range(qh):
                            nc.tensor.matmul(
                                q_ps[m], wq_sb[:, hc, bass.ts(m, 128)], xs,
                                start=st_, stop=sp_,
                            )
                        nc.tensor.matmul(k_ps, wk_sb[:, hc, :], xs,
                                         start=st_, stop=sp_)
                        nc.tensor.matmul(v_ps, wv_sb[:, hc, :], xs,
                                         start=st_, stop=sp_)
                if st == 0:
                    while pending_loads:
                        pending_loads.pop(0)[1]()
                # r = 1/sqrt(mean + eps); fold into cos/sin tables
                sd_sb = statp.tile([1, ST], F32, tag="stat", name="sd_sb")
                nc.scalar.activation(
                    sd_sb, sq_ps, mybir.ActivationFunctionType.Sqrt,
                    bias=eps_sb, scale=1.0 / h,
                )
                rr_sb = statp.tile([1, ST], F32, tag="stat", name="rr_sb")
                nc.vector.reciprocal(rr_sb, sd_sb)
                R_t = tabp.tile([128, ST], F32, tag="R", name="R_t",
                                bufs=1)
                nc.gpsimd.partition_broadcast(R_t, rr_sb)
                cp_t = tabp.tile([128, ST], F32, tag="cp", name="cp_t",
                                 bufs=1)
                nc.vector.tensor_mul(cp_t, cs_sb, R_t)
                sp_t = tabp.tile([128, ST], F32, tag="sp", name="sp_t",
                                 bufs=1)
                nc.vector.tensor_mul(sp_t, sn_sb, R_t)

                # evacuation: fast ACT copy frees the PSUM bank, then
                # norm+RoPE happens SBUF-side on DVE (in place; the u-halves
                # read the raw values before the cos-multiply overwrites)
                def rope_xform(dst):
                    u_sb = ropep.tile([128, ST], F32, tag="u", name="u_sb",
                                      bufs=1)
                    nc.vector.tensor_mul(
                        u_sb[0:64, :], dst[64:128, :], sp_t[64:128, :])
                    nc.vector.tensor_mul(
                        u_sb[64:128, :], dst[0:64, :], sp_t[0:64, :])
                    nc.vector.tensor_mul(dst, dst, cp_t)
                    nc.vector.tensor_add(dst, dst, u_sb)

                if diag == "no_evac":
                    return
                nc.scalar.copy(kT_t[st], k_ps)
                vT_sb = ropep.tile([128, ST], F32, tag="vT", name="vT_sb",
                                   bufs=2)
                nc.vector.tensor_copy(vT_sb, v_ps)
                nc.vector.tensor_mul(vT_sb, vT_sb, R_t)

                def do_transposes(st=st, vT_sb=vT_sb):
                    for jj in range(ST // 128):
                        jc = st * (ST // 128) + jj
                        vt_ps = acc_ps.tile([128, 128], F32, tag="acc",
                                            name="vt_ps")
                        nc.tensor.transpose(vt_ps,
                                            vT_sb[:, bass.ts(jj, 128)],
                                            ident_sb)
                        nc.scalar.copy(vnat_sb[:, jc, :], vt_ps)

                if st + 1 < NST:
                    deferred_tp.append(do_transposes)
                else:
                    do_transposes()
                rope_xform(kT_t[st])
                q_copy = [nc.scalar.copy, nc.vector.tensor_copy,
                          nc.scalar.copy, nc.vector.tensor_copy]
                q_dsts = []
                for m in range(qh):
                    dst = ropep.tile([128, ST], F32, tag="t", name="t_sb",
                                     bufs=5)
                    q_copy[m % len(q_copy)](dst, q_ps[m])
                    q_dsts.append(dst)
                for m in range(qh):
                    rope_xform(q_dsts[m])
                    nc.scalar.copy(qT_t[st][:, m, :], q_dsts[m])

            for st in range(NST):
                pass_b(st)

            if stop_after != "p1":
                for wc in range(h // WOC):
                    nc.sync.dma_start(
                        out=wo_sb[:, :, bass.ts(wc, WOC)],
                        in_=wo_t[:, :, bass.ts(wc, WOC)],
                    )


            # attn^T per i-tile for precise dependency tracking
            attnT_h = [
                [persist.tile([128, 2, ST], BF16, tag=f"aT0{_t}",
                              name=f"attnT01_{_t}") for _t in range(NST)],
                [persist.tile([128, 2, ST], BF16, tag=f"aT1{_t}",
                              name=f"attnT23_{_t}") for _t in range(NST)],
            ]

            def attn_slice(m, ti, cols):
                return attnT_h[m // 2][ti][:, m % 2, cols]

            # ---- phase 3 + 4 interleaved: attention per i-tile (both
            # head pairs), then immediately the o_proj matmuls for that
            # i-range so they fill PE stalls in the next i-tile's attention
            def attn_tile(hp, ti, q_all):
                heads = (2 * hp, 2 * hp + 1)
                iss = bass.ts(ti, ST)
                q_sbs = [q_all[2 * hp], q_all[2 * hp + 1]]
                av_ps = [acc_ps.tile([128, ST], F32, tag="acc",
                                     name=f"av_ps{i}") for i in range(2)]
                z_ps = [acc_ps.tile([1, ST], F32, tag="acc",
                                    name=f"z_ps{i}") for i in range(2)]
                njc = (ti + 1) * (ST // 128)
                for jc in range(njc):
                    st_, sp_ = (jc == 0), (jc == njc - 1)
                    # diagonal chunks: columns left of the 128-wide causal
                    # block (at bcol) are fully masked -> zero them, compute
                    # scores on [dcol, ST) (dcol clamped so the fp32r matmul
                    # stays >=256 wide), exp only on [bcol, ST), and
                    # affine-select the block itself
                    # diagonal chunks: everything (scores, exp, AV, Z)
                    # narrows to the valid columns [bcol, ST); bf16 matmuls
                    # run at full rate at any width.  PSUM accumulation into
                    # av/z starts full-width at jc==0, later chunks
                    # accumulate into the [bcol:] sub-range only (the
                    # skipped columns would add zero).
                    bcol = max(0, jc * 128 - ti * ST)
                    w = ST - bcol
                    for i in range(2):
                        s_ps = acc_ps.tile([128, w], F32, tag="acc",
                                           name=f"s_ps{i}")
                        nc.tensor.matmul(
                            s_ps,
                            kT_t[jc // (ST // 128)][
                                :, bass.ts(jc % (ST // 128), 128)],
                            q_sbs[i][:, bcol:],
                            start=True, stop=True,
                        )
                        p_sb = probs.tile([128, w], BF16, tag="p",
                                          name=f"p_sb{i}", bufs=6)
                        nc.scalar.activation(
                            p_sb, s_ps,
                            mybir.ActivationFunctionType.Exp,
                            scale=scale,
                        )
                        if (jc + 1) * 128 > ti * ST:
                            nc.gpsimd.affine_select(
                                out=p_sb[:, 0:128],
                                in_=p_sb[:, 0:128],
                                pattern=[[1, 128]],
                                compare_op=mybir.AluOpType.is_ge,
                                fill=0.0,
                                base=0,
                                channel_multiplier=-1,
                            )
                        nc.tensor.matmul(av_ps[i][:, bcol:],
                                         vnat_sb[:, jc, :], p_sb,
                                         start=st_, stop=sp_)
                        nc.tensor.matmul(z_ps[i][:, bcol:], ones_bf, p_sb,
                                         start=st_, stop=sp_)
                for i, hh in enumerate(heads):
                    zr_sb = statp.tile([1, ST], F32, tag="stat",
                                       name="zr_sb")
                    nc.vector.reciprocal(zr_sb, z_ps[i])
                    ZR_sb = bcastp.tile([128, ST], F32, tag="bcast",
                                        name="ZR_sb")
                    nc.gpsimd.partition_broadcast(ZR_sb, zr_sb)
                    nc.vector.tensor_mul(attn_slice(hh, ti, slice(None)),
                                         av_ps[i], ZR_sb)

            def o_proj_tile(ti):
                # per i-tile: sc-outer, ht-pair inner; 2 PSUM banks per
                # pair, evacuate into one [128, HTP] tile, single DMA out
                # on the gpsimd SWDGE queue
                for hp in range(NHP):
                    nbl = min(2, NHB - 2 * hp)
                    for sc in range(ti * (ST // 128), (ti + 1) * (ST // 128)):
                        scs = bass.ts(sc, 128)
                        o_ps = [acc_ps.tile([128, 512], F32, tag="acc",
                                            name=f"o_ps{_hh}")
                                for _hh in range(nbl)]
                        for hh in range(nbl):
                            for m in range(qh):
                                nc.tensor.matmul(
                                    o_ps[hh],
                                    attn_slice(m, ti,
                                               bass.ts(sc - ti * (ST // 128),
                                                       128)),
                                    wo_sb[:, m, bass.ts(2 * hp + hh, 512)],
                                    start=(m == 0), stop=(m == qh - 1),
                                )
                        o_sb = outp.tile([128, nbl, 512], F32, tag="o",
                                         name="o_sb", bufs=4)
                        for hh in range(nbl):
                            if (sc + hp + hh) % 2 == 0:
                                nc.scalar.copy(o_sb[:, hh, :], o_ps[hh])
                            else:
                                nc.vector.tensor_copy(o_sb[:, hh, :], o_ps[hh])
                        if ti == NST - 1 and sc % 4 == 3:
                            engs = [nc.sync, nc.scalar, nc.gpsimd]
                            for hh in range(nbl):
                                engs[(2 * hp + hh) % 3].dma_start(
                                    out=out[scs, bass.ts(2 * hp + hh, 512)],
                                    in_=o_sb[:, hh, :],
                                )
                        else:
                            eng = (nc.sync if (sc + hp) % 2 == 0
                                   else nc.gpsimd)
                            eng.dma_start(
                                out=out[scs,
                                        2 * hp * 512:(2 * hp + nbl) * 512],
                                in_=o_sb,
                            )

            if stop_after not in ("p1", "p2"):
                def attn_full_tile(ti):
                    iss = bass.ts(ti, ST)
                    q_all = [qT_t[ti][:, hh, :] for hh in range(qh)]
                    for hp in range(qh // 2):
                        attn_tile(hp, ti, q_all)

                attn_full_tile(0)
                for ti in range(NST):
                    if ti + 1 < NST:
                        attn_full_tile(ti + 1)
                    if stop_after is None:
                        o_proj_tile(ti)

    nc.compile()
    return nc


def make_core_inputs(hidden_states, cos, sin, norm_w, wq, wk, wv, wo,
                     s=S, h=H, qh=QH, n_cores=N_CORES):
    """Host-side sharding + layout preparation. Returns list of in_maps."""
    import ml_dtypes

    dq = qh * HD
    dkv = DKV
    x = np.asarray(hidden_states, dtype=np.float32).reshape(s, h)
    nw = np.asarray(norm_w, dtype=np.float32)
    xT = np.ascontiguousarray(x.T)                      # [h, s]
    cosT = np.ascontiguousarray(np.asarray(cos, np.float32).reshape(s, HD).T)
    sinT = np.ascontiguousarray(np.asarray(sin, np.float32).reshape(s, HD).T)
    # swapped/sign-flipped sin table: rows 0:64 = +sin_half, 64:128 = -sin_half
    sin_half = sinT[0:64]
    sinTs = np.ascontiguousarray(np.concatenate([sinT[64:128], -sin_half], axis=0))
    # fold norm_w into the projection weights
    wq_f = np.asarray(wq, np.float32) * nw[:, None]
    wk_f = np.asarray(wk, np.float32) * nw[:, None]
    wv_f = np.asarray(wv, np.float32) * nw[:, None]
    wo_f = np.asarray(wo, np.float32)

    in_maps = []
    for c in range(n_cores):
        in_maps.append({
            "xT": xT.astype(ml_dtypes.bfloat16),
            "wq": np.ascontiguousarray(
                wq_f[:, c * dq:(c + 1) * dq].astype(ml_dtypes.bfloat16)),
            "wk": np.ascontiguousarray(
                wk_f[:, c * dkv:(c + 1) * dkv].astype(ml_dtypes.bfloat16)),
            "wv": np.ascontiguousarray(
                wv_f[:, c * dkv:(c + 1) * dkv].astype(ml_dtypes.bfloat16)),
            "wo": np.ascontiguousarray(wo_f[c * dq:(c + 1) * dq, :]
                                       .astype(ml_dtypes.bfloat16)),
            "cosT": cosT,
            "sinTs": sinTs,
        })
    return in_maps


_NC_CACHE = {}


def kernel(hidden_states, cos, sin, norm_w, wq, wk, wv, wo):
    from concourse.bass_utils import run_bass_kernel_spmd

    if "nc" not in _NC_CACHE:
        _NC_CACHE["nc"] = build_bass()
    nc = _NC_CACHE["nc"]
    in_maps = make_core_inputs(hidden_states, cos, sin, norm_w, wq, wk, wv, wo)
    res = run_bass_kernel_spmd(nc, in_maps, core_ids=list(range(N_CORES)))
    partials = [m["out"] for m in res.results]
    out = np.asarray(hidden_states, np.float32).reshape(S, H).copy()
    for p in partials:
        out += p
    return out.reshape(B, S, H)


# revision 86
# speedup vs baseline: 1.3617x; 1.0002x over previous
"""Mixtral attention layer (B=1, S=2048, H=4096, NH=32, NKV=8, HD=128) on 8
Trainium2 NeuronCores, tensor-parallel over heads.

Sharding: core c owns 4 query heads + 1 KV head (column-shard of wq/wk/wv,
row-shard of wo).  Each core computes a full [S, H] partial of the o_proj
output; the host sums the 8 partials and adds the residual (the gather of a
row-parallel matmul).

Per-core pipeline (bf16 weights/activations on the PE, fp32 PSUM
accumulation everywhere; sized against the TimelineSim cost model):
  Phase 1 (fused stats + projections): x^T streamed once in bf16 (2-chunk
    group DMAs on the SP queue, weights interleaved in need order); per
    H-chunk the 6 projection matmuls (4 q heads + k + v) accumulate in
    PSUM while x^2 is squared (ACT/DVE) and pair-summed so a ones-vector
    matmul reduces sum(x^2) over H every second chunk.  r =
    1/sqrt(mean+eps) folds into per-tile RoPE cos/sin tables; PSUM
    evacuation does norm+RoPE on DVE with PSUM-freeing copies spread over
    ACT/Pool.  v is transposed to natural layout per tile (transposes
    deferred into the next tile's stream so the stats chain never blocks
    the in-order PE queue).  norm_w is folded into the weights host-side.
  Attention: per head-pair sweep (GQA: both heads share the core's single
    KV head), causal flash-style: scores^T = k^T.T @ q^T per 128-key
    chunk in bf16 (diagonal chunks narrowed to the valid column range),
    exp on ACT, causal mask via affine_select on the 128-wide diagonal
    block, unnormalized AV + ones-matmul row-sum Z accumulate in PSUM;
    1/Z applied at AV evacuation into per-tile SBUF attn^T tiles.
    Attention runs one i-tile ahead of o_proj.
  o_proj: per i-tile, attn^T @ wo (bf16, own SBUF slot, chunk-loaded at
    the phase boundary); [128, 1024] output tiles DMA out alternating the
    SP HWDGE and gpsimd SWDGE queues, the last s-chunk split 512-wide
    across three queues to shorten the drain tail.

q^T/k^T stay SBUF-resident in bf16 (one tile per s-tile); attn^T is
per-i-tile.  All timing tuned against TimelineSim (per-DMA HWDGE cost,
serial DMA_ENGINES device, in-order engine queues, PE p-state ramp).
"""

import math

import numpy as np

import concourse.bass as bass
import concourse.tile as tile
from concourse import bacc, mybir
from concourse.masks import make_identity

F32 = mybir.dt.float32
F32R = mybir.dt.float32r
BF16 = mybir.dt.bfloat16

# Full problem dims
B, S, H, NH, NKV, HD = 1, 2048, 4096, 32, 8, 128
EPS = 1e-5
N_CORES = 8
QH = NH // N_CORES          # query heads per core = 4
DQ = QH * HD                # q columns per core = 512
DKV = (NKV // N_CORES) * HD  # kv columns per core = 128


def build_bass(s=S, h=H, qh=QH, stop_after=None, diag=None):
    """Build the single-core Bass module (same NEFF on all 8 cores)."""
    ST = 512 if s >= 512 else s       # s-tile width (proj + attention i-tiles)
    NST = s // ST                     # number of s-tiles
    HC = h // 128                     # H contraction chunks
    NJ = s // 128                     # j chunks (keys)
    dq = qh * HD
    scale = 1.0 / math.sqrt(HD)
    XG = 2                            # x chunks per DMA group
    WQG = max(1, HC // 8)             # wq chunks per DMA group
    WKG = max(1, HC // 2)             # wk/wv chunks per DMA group
    NHB = h // 512                    # o_proj 512-col blocks
    NHP = (NHB + 1) // 2              # block pairs per sc chunk
    HTP = 1024                        # o_proj ht-pair width (2 PSUM banks)
    WOC = 512                         # wo load chunk width

    nc = bacc.Bacc(None, target_bir_lowering=False)

    xT = nc.dram_tensor("xT", [h, s], BF16, kind="ExternalInput")
    wq = nc.dram_tensor("wq", [h, dq], BF16, kind="ExternalInput")
    wk = nc.dram_tensor("wk", [h, DKV], BF16, kind="ExternalInput")
    wv = nc.dram_tensor("wv", [h, DKV], BF16, kind="ExternalInput")
    wo = nc.dram_tensor("wo", [dq, h], BF16, kind="ExternalInput")
    cosT = nc.dram_tensor("cosT", [HD, s], F32, kind="ExternalInput")
    sinTs = nc.dram_tensor("sinTs", [HD, s], F32, kind="ExternalInput")
    out = nc.dram_tensor("out", [s, h], F32, kind="ExternalOutput")

    xT_t = xT.rearrange("(ho hi) s -> hi ho s", hi=128)
    wq_t = wq.rearrange("(ho hi) d -> hi ho d", hi=128)
    wk_t = wk.rearrange("(ho hi) d -> hi ho d", hi=128)
    wv_t = wv.rearrange("(ho hi) d -> hi ho d", hi=128)
    wo_t = wo.rearrange("(do di) h -> di do h", di=128)

    with tile.TileContext(nc) as tc:
        with (
            tc.tile_pool(name="persist", bufs=1) as persist,
            tc.tile_pool(name="xin", bufs=4) as xin,
                        tc.tile_pool(name="rope", bufs=3) as ropep,
            tc.tile_pool(name="statp", bufs=2) as statp,
            tc.tile_pool(name="tabp", bufs=2) as tabp,
            tc.tile_pool(name="tabin", bufs=1) as tabin,
            tc.tile_pool(name="outp", bufs=4) as outp,
            tc.tile_pool(name="bcastp", bufs=1) as bcastp,
            tc.tile_pool(name="probs", bufs=6) as probs,
            tc.tile_pool(name="acc_ps", bufs=8, space="PSUM") as acc_ps,
        ):
            # ---- persistent SBUF tensors ----
            # Slot reuse chains (same tag, sequential lifetimes):
            #   wq (8MB) -> wo (8MB)         tag "bigw"
            #   wk (2MB) -> attnT heads 0-1  tag "wk"
            #   wv (2MB) -> attnT heads 2-3  tag "wv"
            #   cos (1MB) -> v natural (1MB) tag "cosvnat"
            wq_sb = persist.tile([128, HC, dq], BF16, tag="bigw")
            wo_sb = persist.tile([128, qh, h], BF16, tag="wo")
            wk_sb = persist.tile([128, HC, DKV], BF16, tag="wk")
            wv_sb = persist.tile([128, HC, DKV], BF16, tag="wv")
            ones_f = persist.tile([128, 1], F32, tag="ones_f")
            ones_bf = persist.tile([128, 1], BF16, tag="ones_bf")
            eps_sb = persist.tile([1, 1], F32, tag="eps")
            ident_sb = persist.tile([128, 128], F32, tag="ident")
            kT_t = [persist.tile([128, ST], BF16, tag=f"kT{_t}",
                                 name=f"kT_t{_t}") for _t in range(NST)]
            vnat_sb = persist.tile([128, NJ, 128], BF16, tag="vnat")
            # q^T stays SBUF-resident in bf16 (scores run in bf16),
            # one tile per s-tile for precise dependency tracking
            qT_t = [persist.tile([128, qh, ST], BF16, tag=f"qT{_t}",
                                 name=f"qT_t{_t}") for _t in range(NST)]

            # chunked weight loads: the first two x groups of tile 0 are
            # requested ahead of the weights so the DMA FIFO serves them
            # first; wk/wv before wq (k/v matmuls are ordered first per
            # chunk), wq in small chunks so early q matmuls start fast
            def load_wq(g):
                nc.sync.dma_start(out=wq_sb[:, g * WQG:(g + 1) * WQG, :],
                                  in_=wq_t[:, g * WQG:(g + 1) * WQG, :])

            def load_wkv(g):
                nc.sync.dma_start(out=wk_sb[:, g * WKG:(g + 1) * WKG, :],
                                  in_=wk_t[:, g * WKG:(g + 1) * WKG, :])
                nc.sync.dma_start(out=wv_sb[:, g * WKG:(g + 1) * WKG, :],
                                  in_=wv_t[:, g * WKG:(g + 1) * WKG, :])

            n_wq = HC // WQG
            load_wq(0)
            x_pre = []
            for hg in range(min(3, HC // XG)):
                x_sb = xin.tile([128, XG, ST], BF16, tag="x",
                                name=f"x_pre{hg}", bufs=4)
                nc.sync.dma_start(out=x_sb,
                                  in_=xT_t[:, hg * XG:(hg + 1) * XG,
                                           bass.ts(0, ST)])
                x_pre.append(x_sb)
            load_wkv(0)
            if n_wq > 1:
                load_wq(1)
            # remaining weight-chunk dispatches are spread through tile 0's
            # loop (emitted from pass_b) so the in-order ACT SEQ never sits
            # on a full DMA request queue ahead of the squares
            pending_loads = [(g * WQG // XG, lambda g=g: load_wq(g))
                             for g in range(2 if n_wq > 1 else 1, n_wq)]
            pending_loads += [(g * WKG // XG, lambda g=g: load_wkv(g))
                             for g in range(1, HC // WKG)]
            pending_loads.sort(key=lambda t: t[0])
            nc.vector.memset(ones_f, 1.0)
            nc.vector.tensor_copy(ones_bf, ones_f)
            nc.vector.memset(eps_sb, EPS)
            make_identity(nc, ident_sb)


            # ---- phase 1: fused norm stats + q/k/v projections, one pass
            # over x^T in fp32r ----
            deferred_tp = []

            def pass_b(st):
                ss = bass.ts(st, ST)
                cs_sb = tabin.tile([128, ST], F32, tag="cosin", name="cs_sb")
                sn_sb = tabin.tile([128, ST], F32, tag="sinin", name="sn_sb")

                def load_tabs():
                    nc.sync.dma_start(out=cs_sb, in_=cosT[:, ss])
                    nc.sync.dma_start(out=sn_sb, in_=sinTs[:, ss])

                if st == 0:
                    pending_loads.append((max(0, HC // XG - 6), load_tabs))
                    pending_loads.sort(key=lambda t: t[0])
                else:
                    load_tabs()
                sq_ps = acc_ps.tile([1, ST], F32, tag="acc", name="sq_ps")
                q_ps = [acc_ps.tile([128, ST], F32, tag="acc", name=f"q_ps{m}")
                        for m in range(qh)]
                k_ps = acc_ps.tile([128, ST], F32, tag="acc", name="k_ps")
                v_ps = acc_ps.tile([128, ST], F32, tag="acc", name="v_ps")
                for hg in range(HC // XG):
                    while (st == 0 and pending_loads
                           and pending_loads[0][0] <= hg + 3):
                        pending_loads.pop(0)[1]()
                    if hg == min(2, HC // XG - 1) and deferred_tp:
                        deferred_tp.pop(0)()
                    if st == 0 and hg < len(x_pre):
                        x_sb = x_pre[hg]
                    else:
                        x_sb = xin.tile([128, XG, ST], BF16, tag="x",
                                        name="x_sb", bufs=4)
                        nc.sync.dma_start(out=x_sb,
                                          in_=xT_t[:, hg * XG:(hg + 1) * XG,
                                                   ss])
                    for hi in range(XG):
                        hc = hg * XG + hi
                        xs = x_sb[:, hi, :]
                        st_, sp_ = (hc == 0), (hc == HC - 1)
                        x2_sb = probs.tile([128, ST], BF16, tag="p",
                                           name="x2_sb", bufs=6)
                        if hc % 2 == 0:
                            nc.scalar.square(x2_sb, xs)
                            x2_prev = x2_sb
                        else:
                            nc.vector.tensor_mul(x2_sb, xs, xs)
                            nc.vector.tensor_add(x2_sb, x2_sb, x2_prev)
                            nc.tensor.matmul(sq_ps, ones_bf, x2_sb,
                                             start=(hc == 1),
                                             stop=(hc == HC - 1))
                        for m in range(qh):
                            nc.tensor.matmul(
                                q_ps[m], wq_sb[:, hc, bass.ts(m, 128)], xs,
                                start=st_, stop=sp_,
                            )
                        nc.tensor.matmul(k_ps, wk_sb[:, hc, :], xs,
                                         start=st_, stop=sp_)
                        nc.tensor.matmul(v_ps, wv_sb[:, hc, :], xs,
                                         start=st_, stop=sp_)
                if st == 0:
                    while pending_loads:
                        pending_loads.pop(0)[1]()
                # r = 1/sqrt(mean + eps); fold into cos/sin tables
                sd_sb = statp.tile([1, ST], F32, tag="stat", name="sd_sb")
                nc.scalar.activation(
                    sd_sb, sq_ps, mybir.ActivationFunctionType.Sqrt,
                    bias=eps_sb, scale=1.0 / h,
                )
                rr_sb = statp.tile([1, ST], F32, tag="stat", name="rr_sb")
                nc.vector.reciprocal(rr_sb, sd_sb)
                R_t = tabp.tile([128, ST], F32, tag="R", name="R_t",
                                bufs=1)
                nc.gpsimd.partition_broadcast(R_t, rr_sb)
                cp_t = tabp.tile([128, ST], F32, tag="cp", name="cp_t",
                                 bufs=1)
                nc.vector.tensor_mul(cp_t, cs_sb, R_t)
                sp_t = tabp.tile([128, ST], F32, tag="sp", name="sp_t",
                                 bufs=1)
                nc.vector.tensor_mul(sp_t, sn_sb, R_t)

                # evacuation: fast ACT copy frees the PSUM bank, then
                # norm+RoPE happens SBUF-side on DVE (in place; the u-halves
                # read the raw values before the cos-multiply overwrites)
                def rope_xform(dst):
                    u_sb = ropep.tile([128, ST], F32, tag="u", name="u_sb",
                                      bufs=1)
                    nc.vector.tensor_mul(
                        u_sb[0:64, :], dst[64:128, :], sp_t[64:128, :])
                    nc.vector.tensor_mul(
                        u_sb[64:128, :], dst[0:64, :], sp_t[0:64, :])
                    nc.vector.tensor_mul(dst, dst, cp_t)
                    nc.vector.tensor_add(dst, dst, u_sb)

                if diag == "no_evac":
                    return
                nc.scalar.copy(kT_t[st], k_ps)
                vT_sb = ropep.tile([128, ST], F32, tag="vT", name="vT_sb",
                                   bufs=2)
                nc.vector.tensor_copy(vT_sb, v_ps)
                nc.vector.tensor_mul(vT_sb, vT_sb, R_t)

                def do_transposes(st=st, vT_sb=vT_sb):
                    for jj in range(ST // 128):
                        jc = st * (ST // 128) + jj
                        vt_ps = acc_ps.tile([128, 128], F32, tag="acc",
                                            name="vt_ps")
                        nc.tensor.transpose(vt_ps,
                                            vT_sb[:, bass.ts(jj, 128)],
                                            ident_sb)
                        nc.scalar.copy(vnat_sb[:, jc, :], vt_ps)

                if st + 1 < NST:
                    deferred_tp.append(do_transposes)
                else:
                    do_transposes()
                rope_xform(kT_t[st])
                q_copy = [nc.scalar.copy, nc.vector.tensor_copy,
                          nc.scalar.copy, nc.vector.tensor_copy]
                q_dsts = []
                for m in range(qh):
                    dst = ropep.tile([128, ST], F32, tag="t", name="t_sb",
                                     bufs=5)
                    q_copy[m % len(q_copy)](dst, q_ps[m])
                    q_dsts.append(dst)
                for m in range(qh):
                    rope_xform(q_dsts[m])
                    nc.scalar.copy(qT_t[st][:, m, :], q_dsts[m])

            for st in range(NST):
                pass_b(st)

            if stop_after != "p1":
                for wc in range(h // WOC):
                    nc.sync.dma_start(
                        out=wo_sb[:, :, bass.ts(wc, WOC)],
                        in_=wo_t[:, :, bass.ts(wc, WOC)],
                    )


            # attn^T per i-tile for precise dependency tracking
            attnT_h = [
                [persist.tile([128, 2, ST], BF16, tag=f"aT0{_t}",
                              name=f"attnT01_{_t}") for _t in range(NST)],
                [persist.tile([128, 2, ST], BF16, tag=f"aT1{_t}",
                              name=f"attnT23_{_t}") for _t in range(NST)],
            ]

            def attn_slice(m, ti, cols):
                return attnT_h[m // 2][ti][:, m % 2, cols]

            # ---- phase 3 + 4 interleaved: attention per i-tile (both
            # head pairs), then immediately the o_proj matmuls for that
            # i-range so they fill PE stalls in the next i-tile's attention
            def attn_tile(hp, ti, q_all):
                heads = (2 * hp, 2 * hp + 1)
                iss = bass.ts(ti, ST)
                q_sbs = [q_all[2 * hp], q_all[2 * hp + 1]]
                av_ps = [acc_ps.tile([128, ST], F32, tag="acc",
                                     name=f"av_ps{i}") for i in range(2)]
                z_ps = [acc_ps.tile([1, ST], F32, tag="acc",
                                    name=f"z_ps{i}") for i in range(2)]
                njc = (ti + 1) * (ST // 128)
                for jc in range(njc):
                    st_, sp_ = (jc == 0), (jc == njc - 1)
                    # diagonal chunks: columns left of the 128-wide causal
                    # block (at bcol) are fully masked -> zero them, compute
                    # scores on [dcol, ST) (dcol clamped so the fp32r matmul
                    # stays >=256 wide), exp only on [bcol, ST), and
                    # affine-select the block itself
                    # diagonal chunks: everything (scores, exp, AV, Z)
                    # narrows to the valid columns [bcol, ST); bf16 matmuls
                    # run at full rate at any width.  PSUM accumulation into
                    # av/z starts full-width at jc==0, later chunks
                    # accumulate into the [bcol:] sub-range only (the
                    # skipped columns would add zero).
                    bcol = max(0, jc * 128 - ti * ST)
                    w = ST - bcol
                    for i in range(2):
                        s_ps = acc_ps.tile([128, w], F32, tag="acc",
                                           name=f"s_ps{i}")
                        nc.tensor.matmul(
                            s_ps,
                            kT_t[jc // (ST // 128)][
                                :, bass.ts(jc % (ST // 128), 128)],
                            q_sbs[i][:, bcol:],
                            start=True, stop=True,
                        )
                        p_sb = probs.tile([128, w], BF16, tag="p",
                                          name=f"p_sb{i}", bufs=6)
                        nc.scalar.activation(
                            p_sb, s_ps,
                            mybir.ActivationFunctionType.Exp,
                            scale=scale,
                        )
                        if (jc + 1) * 128 > ti * ST:
                            nc.gpsimd.affine_select(
                                out=p_sb[:, 0:128],
                                in_=p_sb[:, 0:128],
                                pattern=[[1, 128]],
                                compare_op=mybir.AluOpType.is_ge,
                                fill=0.0,
                                base=0,
                                channel_multiplier=-1,
                            )
                        nc.tensor.matmul(av_ps[i][:, bcol:],
                                         vnat_sb[:, jc, :], p_sb,
                                         start=st_, stop=sp_)
                        nc.tensor.matmul(z_ps[i][:, bcol:], ones_bf, p_sb,
                                         start=st_, stop=sp_)
                for i, hh in enumerate(heads):
                    zr_sb = statp.tile([1, ST], F32, tag="stat",
                                       name="zr_sb")
                    nc.vector.reciprocal(zr_sb, z_ps[i])
                    ZR_sb = bcastp.tile([128, ST], F32, tag="bcast",
                                        name="ZR_sb")
                    nc.gpsimd.partition_broadcast(ZR_sb, zr_sb)
                    nc.vector.tensor_mul(attn_slice(hh, ti, slice(None)),
                                         av_ps[i], ZR_sb)

            def o_proj_tile(ti):
                # per i-tile: sc-outer, ht-pair inner; 2 PSUM banks per
                # pair, evacuate into one [128, HTP] tile, single DMA out
                # on the gpsimd SWDGE queue
                for hp in range(NHP):
                    nbl = min(2, NHB - 2 * hp)
                    for sc in range(ti * (ST // 128), (ti + 1) * (ST // 128)):
                        scs = bass.ts(sc, 128)
                        o_ps = [acc_ps.tile([128, 512], F32, tag="acc",
                                            name=f"o_ps{_hh}")
                                for _hh in range(nbl)]
                        for hh in range(nbl):
                            for m in range(qh):
                                nc.tensor.matmul(
                                    o_ps[hh],
                                    attn_slice(m, ti,
                                               bass.ts(sc - ti * (ST // 128),
                                                       128)),
                                    wo_sb[:, m, bass.ts(2 * hp + hh, 512)],
                                    start=(m == 0), stop=(m == qh - 1),
                                )
                        o_sb = outp.tile([128, nbl, 512], F32, tag="o",
                                         name="o_sb", bufs=4)
                        for hh in range(nbl):
                            nc.vector.tensor_copy(o_sb[:, hh, :], o_ps[hh])
                        if ti == NST - 1 and sc % 4 == 3:
                            engs = [nc.sync, nc.scalar, nc.gpsimd]
                            for hh in range(nbl):
                                engs[(2 * hp + hh) % 3].dma_start(
                                    out=out[scs, bass.ts(2 * hp + hh, 512)],
                                    in_=o_sb[:, hh, :],
                                )
                        else:
                            eng = (nc.sync if (sc + hp) % 2 == 0
                                   else nc.gpsimd)
                            eng.dma_start(
                                out=out[scs,
                                        2 * hp * 512:(2 * hp + nbl) * 512],
                                in_=o_sb,
                            )

            if stop_after not in ("p1", "p2"):
                def attn_full_tile(ti):
                    iss = bass.ts(ti, ST)
                    q_all = [qT_t[ti][:, hh, :] for hh in range(qh)]
                    for hp in range(qh // 2):
                        attn_tile(hp, ti, q_all)

                attn_full_tile(0)
                for ti in range(NST):
                    if ti + 1 < NST:
                        attn_full_tile(ti + 1)
                    if stop_after is None:
                        o_proj_tile(ti)

    nc.compile()
    return nc


def make_core_inputs(hidden_states, cos, sin, norm_w, wq, wk, wv, wo,
                     s=S, h=H, qh=QH, n_cores=N_CORES):
    """Host-side sharding + layout preparation. Returns list of in_maps."""
    import ml_dtypes

    dq = qh * HD
    dkv = DKV
    x = np.asarray(hidden_states, dtype=np.float32).reshape(s, h)
    nw = np.asarray(norm_w, dtype=np.float32)
    xT = np.ascontiguousarray(x.T)                      # [h, s]
    cosT = np.ascontiguousarray(np.asarray(cos, np.float32).reshape(s, HD).T)
    sinT = np.ascontiguousarray(np.asarray(sin, np.float32).reshape(s, HD).T)
    # swapped/sign-flipped sin table: rows 0:64 = +sin_half, 64:128 = -sin_half
    sin_half = sinT[0:64]
    sinTs = np.ascontiguousarray(np.concatenate([sinT[64:128], -sin_half], axis=0))
    # fold norm_w into the projection weights
    wq_f = np.asarray(wq, np.float32) * nw[:, None]
    wk_f = np.asarray(wk, np.float32) * nw[:, None]
    wv_f = np.asarray(wv, np.float32) * nw[:, None]
    wo_f = np.asarray(wo, np.float32)

    in_maps = []
    for c in range(n_cores):
        in_maps.append({
            "xT": xT.astype(ml_dtypes.bfloat16),
            "wq": np.ascontiguousarray(
                wq_f[:, c * dq:(c + 1) * dq].astype(ml_dtypes.bfloat16)),
            "wk": np.ascontiguousarray(
                wk_f[:, c * dkv:(c + 1) * dkv].astype(ml_dtypes.bfloat16)),
            "wv": np.ascontiguousarray(
                wv_f[:, c * dkv:(c + 1) * dkv].astype(ml_dtypes.bfloat16)),
            "wo": np.ascontiguousarray(wo_f[c * dq:(c + 1) * dq, :]
                                       .astype(ml_dtypes.bfloat16)),
            "cosT": cosT,
            "sinTs": sinTs,
        })
    return in_maps


_NC_CACHE = {}


def kernel(hidden_states, cos, sin, norm_w, wq, wk, wv, wo):
    from concourse.bass_utils import run_bass_kernel_spmd

    if "nc" not in _NC_CACHE:
        _NC_CACHE["nc"] = build_bass()
    nc = _NC_CACHE["nc"]
    in_maps = make_core_inputs(hidden_states, cos, sin, norm_w, wq, wk, wv, wo)
    res = run_bass_kernel_spmd(nc, in_maps, core_ids=list(range(N_CORES)))
    partials = [m["out"] for m in res.results]
    out = np.asarray(hidden_states, np.float32).reshape(S, H).copy()
    for p in partials:
        out += p
    return out.reshape(B, S, H)


# revision 90
# speedup vs baseline: 1.3847x; 1.0168x over previous
"""Mixtral attention layer (B=1, S=2048, H=4096, NH=32, NKV=8, HD=128) on 8
Trainium2 NeuronCores, tensor-parallel over heads.

Sharding: core c owns 4 query heads + 1 KV head (column-shard of wq/wk/wv,
row-shard of wo).  Each core computes a full [S, H] partial of the o_proj
output; the host sums the 8 partials and adds the residual (the gather of a
row-parallel matmul).

Per-core pipeline (bf16 weights/activations on the PE, fp32 PSUM
accumulation everywhere; sized against the TimelineSim cost model):
  Phase 1 (fused stats + projections): x^T streamed once in bf16 (2-chunk
    group DMAs on the SP queue, weights interleaved in need order); per
    H-chunk the 6 projection matmuls (4 q heads + k + v) accumulate in
    PSUM while x^2 is squared (ACT/DVE) and pair-summed so a ones-vector
    matmul reduces sum(x^2) over H every second chunk.  r =
    1/sqrt(mean+eps) folds into per-tile RoPE cos/sin tables; PSUM
    evacuation does norm+RoPE on DVE with PSUM-freeing copies spread over
    ACT/Pool.  v is transposed to natural layout per tile (transposes
    deferred into the next tile's stream so the stats chain never blocks
    the in-order PE queue).  norm_w is folded into the weights host-side.
  Attention: per head-pair sweep (GQA: both heads share the core's single
    KV head), causal flash-style: scores^T = k^T.T @ q^T per 128-key
    chunk in bf16 (diagonal chunks narrowed to the valid column range),
    exp on ACT, causal mask via affine_select on the 128-wide diagonal
    block, unnormalized AV + ones-matmul row-sum Z accumulate in PSUM;
    1/Z applied at AV evacuation into per-tile SBUF attn^T tiles.
    Attention runs one i-tile ahead of o_proj.
  o_proj: per i-tile, attn^T @ wo (bf16, own SBUF slot, chunk-loaded at
    the phase boundary); [128, 1024] output tiles DMA out alternating the
    SP HWDGE and gpsimd SWDGE queues, the last s-chunk split 512-wide
    across three queues to shorten the drain tail.

q^T/k^T stay SBUF-resident in bf16 (one tile per s-tile); attn^T is
per-i-tile.  All timing tuned against TimelineSim (per-DMA HWDGE cost,
serial DMA_ENGINES device, in-order engine queues, PE p-state ramp).
"""

import math

import numpy as np

import concourse.bass as bass
import concourse.tile as tile
from concourse import bacc, mybir
from concourse.masks import make_identity

F32 = mybir.dt.float32
F32R = mybir.dt.float32r
BF16 = mybir.dt.bfloat16

# Full problem dims
B, S, H, NH, NKV, HD = 1, 2048, 4096, 32, 8, 128
EPS = 1e-5
N_CORES = 8
QH = NH // N_CORES          # query heads per core = 4
DQ = QH * HD                # q columns per core = 512
DKV = (NKV // N_CORES) * HD  # kv columns per core = 128


def build_bass(s=S, h=H, qh=QH, stop_after=None, diag=None):
    """Build the single-core Bass module (same NEFF on all 8 cores)."""
    ST = 512 if s >= 512 else s       # s-tile width (proj + attention i-tiles)
    NST = s // ST                     # number of s-tiles
    HC = h // 128                     # H contraction chunks
    NJ = s // 128                     # j chunks (keys)
    dq = qh * HD
    scale = 1.0 / math.sqrt(HD)
    XG = 2                            # x chunks per DMA group
    WQG = max(1, HC // 8)             # wq chunks per DMA group
    WKG = max(1, HC // 2)             # wk/wv chunks per DMA group
    NHB = h // 512                    # o_proj 512-col blocks
    NHP = (NHB + 1) // 2              # block pairs per sc chunk
    HTP = 1024                        # o_proj ht-pair width (2 PSUM banks)
    WOC = 512                         # wo load chunk width

    nc = bacc.Bacc(None, target_bir_lowering=False)

    xT = nc.dram_tensor("xT", [h, s], BF16, kind="ExternalInput")
    wq = nc.dram_tensor("wq", [h, dq], BF16, kind="ExternalInput")
    wk = nc.dram_tensor("wk", [h, DKV], BF16, kind="ExternalInput")
    wv = nc.dram_tensor("wv", [h, DKV], BF16, kind="ExternalInput")
    wo = nc.dram_tensor("wo", [dq, h], BF16, kind="ExternalInput")
    cosT = nc.dram_tensor("cosT", [HD, s], F32, kind="ExternalInput")
    sinTs = nc.dram_tensor("sinTs", [HD, s], F32, kind="ExternalInput")
    out = nc.dram_tensor("out", [s, h], F32, kind="ExternalOutput")

    xT_t = xT.rearrange("(ho hi) s -> hi ho s", hi=128)
    wq_t = wq.rearrange("(ho hi) d -> hi ho d", hi=128)
    wk_t = wk.rearrange("(ho hi) d -> hi ho d", hi=128)
    wv_t = wv.rearrange("(ho hi) d -> hi ho d", hi=128)
    wo_t = wo.rearrange("(do di) h -> di do h", di=128)

    with tile.TileContext(nc) as tc:
        with (
            tc.tile_pool(name="persist", bufs=1) as persist,
            tc.tile_pool(name="xin", bufs=4) as xin,
                        tc.tile_pool(name="rope", bufs=3) as ropep,
            tc.tile_pool(name="statp", bufs=2) as statp,
            tc.tile_pool(name="tabp", bufs=2) as tabp,
            tc.tile_pool(name="tabin", bufs=1) as tabin,
            tc.tile_pool(name="outp", bufs=4) as outp,
            tc.tile_pool(name="bcastp", bufs=1) as bcastp,
            tc.tile_pool(name="probs", bufs=6) as probs,
            tc.tile_pool(name="acc_ps", bufs=8, space="PSUM") as acc_ps,
        ):
            # ---- persistent SBUF tensors ----
            # Slot reuse chains (same tag, sequential lifetimes):
            #   wq (8MB) -> wo (8MB)         tag "bigw"
            #   wk (2MB) -> attnT heads 0-1  tag "wk"
            #   wv (2MB) -> attnT heads 2-3  tag "wv"
            #   cos (1MB) -> v natural (1MB) tag "cosvnat"
            wq_sb = persist.tile([128, HC, dq], BF16, tag="bigw")
            wo_sb = persist.tile([128, qh, h], BF16, tag="wo")
            wk_sb = persist.tile([128, HC, DKV], BF16, tag="wk")
            wv_sb = persist.tile([128, HC, DKV], BF16, tag="wv")
            ones_f = persist.tile([128, 1], F32, tag="ones_f")
            ones_bf = persist.tile([128, 1], BF16, tag="ones_bf")
            eps_sb = persist.tile([1, 1], F32, tag="eps")
            ident_sb = persist.tile([128, 128], F32, tag="ident")
            kT_t = [persist.tile([128, ST], BF16, tag=f"kT{_t}",
                                 name=f"kT_t{_t}") for _t in range(NST)]
            vnat_sb = persist.tile([128, NJ, 128], BF16, tag="vnat")
            # q^T stays SBUF-resident in bf16 (scores run in bf16),
            # one tile per s-tile for precise dependency tracking
            qT_t = [persist.tile([128, qh, ST], BF16, tag=f"qT{_t}",
                                 name=f"qT_t{_t}") for _t in range(NST)]

            # chunked weight loads: the first two x groups of tile 0 are
            # requested ahead of the weights so the DMA FIFO serves them
            # first; wk/wv before wq (k/v matmuls are ordered first per
            # chunk), wq in small chunks so early q matmuls start fast
            def load_wq(g):
                nc.sync.dma_start(out=wq_sb[:, g * WQG:(g + 1) * WQG, :],
                                  in_=wq_t[:, g * WQG:(g + 1) * WQG, :])

            def load_wkv(g):
                nc.sync.dma_start(out=wk_sb[:, g * WKG:(g + 1) * WKG, :],
                                  in_=wk_t[:, g * WKG:(g + 1) * WKG, :])
                nc.sync.dma_start(out=wv_sb[:, g * WKG:(g + 1) * WKG, :],
                                  in_=wv_t[:, g * WKG:(g + 1) * WKG, :])

            n_wq = HC // WQG
            load_wq(0)
            x_pre = []
            for hg in range(min(3, HC // XG)):
                x_sb = xin.tile([128, XG, ST], BF16, tag="x",
                                name=f"x_pre{hg}", bufs=4)
                nc.sync.dma_start(out=x_sb,
                                  in_=xT_t[:, hg * XG:(hg + 1) * XG,
                                           bass.ts(0, ST)])
                x_pre.append(x_sb)
            load_wkv(0)
            if n_wq > 1:
                load_wq(1)
            # remaining weight-chunk dispatches are spread through tile 0's
            # loop (emitted from pass_b) so the in-order ACT SEQ never sits
            # on a full DMA request queue ahead of the squares
            pending_loads = [(g * WQG // XG, lambda g=g: load_wq(g))
                             for g in range(2 if n_wq > 1 else 1, n_wq)]
            pending_loads += [(g * WKG // XG, lambda g=g: load_wkv(g))
                             for g in range(1, HC // WKG)]
            pending_loads.sort(key=lambda t: t[0])
            nc.vector.memset(ones_f, 1.0)
            nc.vector.tensor_copy(ones_bf, ones_f)
            nc.vector.memset(eps_sb, EPS)
            make_identity(nc, ident_sb)


            # ---- phase 1: fused norm stats + q/k/v projections, one pass
            # over x^T in fp32r ----
            deferred_tp = []

            def pass_b(st):
                ss = bass.ts(st, ST)
                cs_sb = tabin.tile([128, ST], F32, tag="cosin", name="cs_sb")
                sn_sb = tabin.tile([128, ST], F32, tag="sinin", name="sn_sb")

                def load_tabs():
                    nc.sync.dma_start(out=cs_sb, in_=cosT[:, ss])
                    nc.sync.dma_start(out=sn_sb, in_=sinTs[:, ss])

                if st == 0:
                    pending_loads.append((max(0, HC // XG - 6), load_tabs))
                    pending_loads.sort(key=lambda t: t[0])
                else:
                    load_tabs()
                sq_ps = acc_ps.tile([1, ST], F32, tag="acc", name="sq_ps")
                q_ps = [acc_ps.tile([128, ST], F32, tag="acc", name=f"q_ps{m}")
                        for m in range(qh)]
                k_ps = acc_ps.tile([128, ST], F32, tag="acc", name="k_ps")
                v_ps = acc_ps.tile([128, ST], F32, tag="acc", name="v_ps")
                for hg in range(HC // XG):
                    while (st == 0 and pending_loads
                           and pending_loads[0][0] <= hg + 3):
                        pending_loads.pop(0)[1]()
                    if hg == min(2, HC // XG - 1) and deferred_tp:
                        deferred_tp.pop(0)()
                    if st == 0 and hg < len(x_pre):
                        x_sb = x_pre[hg]
                    else:
                        x_sb = xin.tile([128, XG, ST], BF16, tag="x",
                                        name="x_sb", bufs=4)
                        nc.sync.dma_start(out=x_sb,
                                          in_=xT_t[:, hg * XG:(hg + 1) * XG,
                                                   ss])
                    for hi in range(XG):
                        hc = hg * XG + hi
                        xs = x_sb[:, hi, :]
                        st_, sp_ = (hc == 0), (hc == HC - 1)
                        x2_sb = probs.tile([128, ST], BF16, tag="p",
                                           name="x2_sb", bufs=6)
                        if hc % 2 == 0:
                            nc.scalar.square(x2_sb, xs)
                            x2_prev = x2_sb
                        else:
                            nc.vector.tensor_mul(x2_sb, xs, xs)
                            nc.vector.tensor_add(x2_sb, x2_sb, x2_prev)
                            nc.tensor.matmul(sq_ps, ones_bf, x2_sb,
                                             start=(hc == 1),
                                             stop=(hc == HC - 1))
                        for m in range(qh):
                            nc.tensor.matmul(
                                q_ps[m], wq_sb[:, hc, bass.ts(m, 128)], xs,
                                start=st_, stop=sp_,
                            )
                        nc.tensor.matmul(k_ps, wk_sb[:, hc, :], xs,
                                         start=st_, stop=sp_)
                        nc.tensor.matmul(v_ps, wv_sb[:, hc, :], xs,
                                         start=st_, stop=sp_)
                if st == 0:
                    while pending_loads:
                        pending_loads.pop(0)[1]()
                # r = 1/sqrt(mean + eps); fold into cos/sin tables
                sd_sb = statp.tile([1, ST], F32, tag="stat", name="sd_sb")
                nc.scalar.activation(
                    sd_sb, sq_ps, mybir.ActivationFunctionType.Sqrt,
                    bias=eps_sb, scale=1.0 / h,
                )
                rr_sb = statp.tile([1, ST], F32, tag="stat", name="rr_sb")
                nc.vector.reciprocal(rr_sb, sd_sb)
                R_t = tabp.tile([128, ST], F32, tag="R", name="R_t",
                                bufs=1)
                nc.gpsimd.partition_broadcast(R_t, rr_sb)
                cp_t = tabp.tile([128, ST], F32, tag="cp", name="cp_t",
                                 bufs=1)
                nc.vector.tensor_mul(cp_t, cs_sb, R_t)
                sp_t = tabp.tile([128, ST], F32, tag="sp", name="sp_t",
                                 bufs=1)
                nc.vector.tensor_mul(sp_t, sn_sb, R_t)

                # evacuation: fast ACT copy frees the PSUM bank, then
                # norm+RoPE happens SBUF-side on DVE (in place; the u-halves
                # read the raw values before the cos-multiply overwrites)
                def rope_xform(dst):
                    u_sb = ropep.tile([128, ST], F32, tag="u", name="u_sb",
                                      bufs=1)
                    nc.vector.tensor_mul(
                        u_sb[0:64, :], dst[64:128, :], sp_t[64:128, :])
                    nc.vector.tensor_mul(
                        u_sb[64:128, :], dst[0:64, :], sp_t[0:64, :])
                    nc.vector.tensor_mul(dst, dst, cp_t)
                    nc.vector.tensor_add(dst, dst, u_sb)

                if diag == "no_evac":
                    return
                nc.scalar.copy(kT_t[st], k_ps)
                vT_sb = ropep.tile([128, ST], F32, tag="vT", name="vT_sb",
                                   bufs=2)
                nc.vector.tensor_copy(vT_sb, v_ps)
                nc.vector.tensor_mul(vT_sb, vT_sb, R_t)

                def do_transposes(st=st, vT_sb=vT_sb):
                    for jj in range(ST // 128):
                        jc = st * (ST // 128) + jj
                        vt_ps = acc_ps.tile([128, 128], F32, tag="acc",
                                            name="vt_ps")
                        nc.tensor.transpose(vt_ps,
                                            vT_sb[:, bass.ts(jj, 128)],
                                            ident_sb)
                        nc.scalar.copy(vnat_sb[:, jc, :], vt_ps)

                if st + 1 < NST:
                    deferred_tp.append(do_transposes)
                else:
                    do_transposes()
                rope_xform(kT_t[st])
                q_copy = [nc.scalar.copy, nc.vector.tensor_copy,
                          nc.scalar.copy, nc.vector.tensor_copy]
                q_dsts = []
                for m in range(qh):
                    dst = ropep.tile([128, ST], F32, tag="t", name="t_sb",
                                     bufs=5)
                    q_copy[m % len(q_copy)](dst, q_ps[m])
                    q_dsts.append(dst)
                for m in range(qh):
                    rope_xform(q_dsts[m])
                    nc.scalar.copy(qT_t[st][:, m, :], q_dsts[m])

            for st in range(NST):
                pass_b(st)

            if stop_after != "p1":
                for wc in range(h // WOC):
                    nc.sync.dma_start(
                        out=wo_sb[:, :, bass.ts(wc, WOC)],
                        in_=wo_t[:, :, bass.ts(wc, WOC)],
                    )


            # attn^T per i-tile for precise dependency tracking
            attnT_h = [
                [persist.tile([128, 2, ST], BF16, tag=f"aT0{_t}",
                              name=f"attnT01_{_t}") for _t in range(NST)],
                [persist.tile([128, 2, ST], BF16, tag=f"aT1{_t}",
                              name=f"attnT23_{_t}") for _t in range(NST)],
            ]

            def attn_slice(m, ti, cols):
                return attnT_h[m // 2][ti][:, m % 2, cols]

            # ---- phase 3 + 4 interleaved: attention per i-tile (both
            # head pairs), then immediately the o_proj matmuls for that
            # i-range so they fill PE stalls in the next i-tile's attention
            def attn_tile(hp, ti, q_all):
                heads = (2 * hp, 2 * hp + 1)
                iss = bass.ts(ti, ST)
                q_sbs = [q_all[2 * hp], q_all[2 * hp + 1]]
                av_ps = [acc_ps.tile([128, ST], F32, tag="acc",
                                     name=f"av_ps{i}") for i in range(2)]
                z_ps = [acc_ps.tile([1, ST], F32, tag="acc",
                                    name=f"z_ps{i}") for i in range(2)]
                njc = (ti + 1) * (ST // 128)
                for jc in range(njc):
                    st_, sp_ = (jc == 0), (jc == njc - 1)
                    # diagonal chunks: columns left of the 128-wide causal
                    # block (at bcol) are fully masked -> zero them, compute
                    # scores on [dcol, ST) (dcol clamped so the fp32r matmul
                    # stays >=256 wide), exp only on [bcol, ST), and
                    # affine-select the block itself
                    # diagonal chunks: everything (scores, exp, AV, Z)
                    # narrows to the valid columns [bcol, ST); bf16 matmuls
                    # run at full rate at any width.  PSUM accumulation into
                    # av/z starts full-width at jc==0, later chunks
                    # accumulate into the [bcol:] sub-range only (the
                    # skipped columns would add zero).
                    bcol = max(0, jc * 128 - ti * ST)
                    w = ST - bcol
                    s_list, p_list = [], []
                    for i in range(2):
                        s_ps = acc_ps.tile([128, w], F32, tag="acc",
                                           name=f"s_ps{i}")
                        nc.tensor.matmul(
                            s_ps,
                            kT_t[jc // (ST // 128)][
                                :, bass.ts(jc % (ST // 128), 128)],
                            q_sbs[i][:, bcol:],
                            start=True, stop=True,
                        )
                        s_list.append(s_ps)
                    for i in range(2):
                        p_sb = probs.tile([128, w], BF16, tag="p",
                                          name=f"p_sb{i}", bufs=6)
                        nc.scalar.activation(
                            p_sb, s_list[i],
                            mybir.ActivationFunctionType.Exp,
                            scale=scale,
                        )
                        if (jc + 1) * 128 > ti * ST:
                            nc.gpsimd.affine_select(
                                out=p_sb[:, 0:128],
                                in_=p_sb[:, 0:128],
                                pattern=[[1, 128]],
                                compare_op=mybir.AluOpType.is_ge,
                                fill=0.0,
                                base=0,
                                channel_multiplier=-1,
                            )
                        p_list.append(p_sb)
                    for i in range(2):
                        nc.tensor.matmul(av_ps[i][:, bcol:],
                                         vnat_sb[:, jc, :], p_list[i],
                                         start=st_, stop=sp_)
                    for i in range(2):
                        nc.tensor.matmul(z_ps[i][:, bcol:], ones_bf,
                                         p_list[i],
                                         start=st_, stop=sp_)
                for i, hh in enumerate(heads):
                    zr_sb = statp.tile([1, ST], F32, tag="stat",
                                       name="zr_sb")
                    nc.vector.reciprocal(zr_sb, z_ps[i])
                    ZR_sb = bcastp.tile([128, ST], F32, tag="bcast",
                                        name="ZR_sb")
                    nc.gpsimd.partition_broadcast(ZR_sb, zr_sb)
                    nc.vector.tensor_mul(attn_slice(hh, ti, slice(None)),
                                         av_ps[i], ZR_sb)

            def o_proj_tile(ti):
                # per i-tile: sc-outer, ht-pair inner; 2 PSUM banks per
                # pair, evacuate into one [128, HTP] tile, single DMA out
                # on the gpsimd SWDGE queue
                for hp in range(NHP):
                    nbl = min(2, NHB - 2 * hp)
                    for sc in range(ti * (ST // 128), (ti + 1) * (ST // 128)):
                        scs = bass.ts(sc, 128)
                        o_ps = [acc_ps.tile([128, 512], F32, tag="acc",
                                            name=f"o_ps{_hh}")
                                for _hh in range(nbl)]
                        for hh in range(nbl):
                            for m in range(qh):
                                nc.tensor.matmul(
                                    o_ps[hh],
                                    attn_slice(m, ti,
                                               bass.ts(sc - ti * (ST // 128),
                                                       128)),
                                    wo_sb[:, m, bass.ts(2 * hp + hh, 512)],
                                    start=(m == 0), stop=(m == qh - 1),
                                )
                        o_sb = outp.tile([128, nbl, 512], F32, tag="o",
                                         name="o_sb", bufs=4)
                        for hh in range(nbl):
                            nc.vector.tensor_copy(o_sb[:, hh, :], o_ps[hh])
                        if ti == NST - 1 and sc % 4 == 3:
                            engs = [nc.sync, nc.scalar, nc.gpsimd]
                            for hh in range(nbl):
                                engs[(2 * hp + hh) % 3].dma_start(
                                    out=out[scs, bass.ts(2 * hp + hh, 512)],
                                    in_=o_sb[:, hh, :],
                                )
                        else:
                            eng = (nc.sync if (sc + hp) % 2 == 0
                                   else nc.gpsimd)
                            eng.dma_start(
                                out=out[scs,
                                        2 * hp * 512:(2 * hp + nbl) * 512],
                                in_=o_sb,
                            )

            if stop_after not in ("p1", "p2"):
                def attn_full_tile(ti):
                    iss = bass.ts(ti, ST)
                    q_all = [qT_t[ti][:, hh, :] for hh in range(qh)]
                    for hp in range(qh // 2):
                        attn_tile(hp, ti, q_all)

                attn_full_tile(0)
                for ti in range(NST):
                    if ti + 1 < NST:
                        attn_full_tile(ti + 1)
                    if stop_after is None:
                        o_proj_tile(ti)

    nc.compile()
    return nc


def make_core_inputs(hidden_states, cos, sin, norm_w, wq, wk, wv, wo,
                     s=S, h=H, qh=QH, n_cores=N_CORES):
    """Host-side sharding + layout preparation. Returns list of in_maps."""
    import ml_dtypes

    dq = qh * HD
    dkv = DKV
    x = np.asarray(hidden_states, dtype=np.float32).reshape(s, h)
    nw = np.asarray(norm_w, dtype=np.float32)
    xT = np.ascontiguousarray(x.T)                      # [h, s]
    cosT = np.ascontiguousarray(np.asarray(cos, np.float32).reshape(s, HD).T)
    sinT = np.ascontiguousarray(np.asarray(sin, np.float32).reshape(s, HD).T)
    # swapped/sign-flipped sin table: rows 0:64 = +sin_half, 64:128 = -sin_half
    sin_half = sinT[0:64]
    sinTs = np.ascontiguousarray(np.concatenate([sinT[64:128], -sin_half], axis=0))
    # fold norm_w into the projection weights
    wq_f = np.asarray(wq, np.float32) * nw[:, None]
    wk_f = np.asarray(wk, np.float32) * nw[:, None]
    wv_f = np.asarray(wv, np.float32) * nw[:, None]
    wo_f = np.asarray(wo, np.float32)

    in_maps = []
    for c in range(n_cores):
        in_maps.append({
            "xT": xT.astype(ml_dtypes.bfloat16),
            "wq": np.ascontiguousarray(
                wq_f[:, c * dq:(c + 1) * dq].astype(ml_dtypes.bfloat16)),
            "wk": np.ascontiguousarray(
                wk_f[:, c * dkv:(c + 1) * dkv].astype(ml_dtypes.bfloat16)),
            "wv": np.ascontiguousarray(
                wv_f[:, c * dkv:(c + 1) * dkv].astype(ml_dtypes.bfloat16)),
            "wo": np.ascontiguousarray(wo_f[c * dq:(c + 1) * dq, :]
                                       .astype(ml_dtypes.bfloat16)),
            "cosT": cosT,
            "sinTs": sinTs,
        })
    return in_maps


_NC_CACHE = {}


def kernel(hidden_states, cos, sin, norm_w, wq, wk, wv, wo):
    from concourse.bass_utils import run_bass_kernel_spmd

    if "nc" not in _NC_CACHE:
        _NC_CACHE["nc"] = build_bass()
    nc = _NC_CACHE["nc"]
    in_maps = make_core_inputs(hidden_states, cos, sin, norm_w, wq, wk, wv, wo)
    res = run_bass_kernel_spmd(nc, in_maps, core_ids=list(range(N_CORES)))
    partials = [m["out"] for m in res.results]
    out = np.asarray(hidden_states, np.float32).reshape(S, H).copy()
    for p in partials:
        out += p
    return out.reshape(B, S, H)
